# revision 22
# baseline (speedup 1.0000x reference)
"""GIN discriminator (4-layer GINConv + global mean pool + sigmoid) on 8 trn2 cores.

Sharding: nodes are split contiguously across 8 cores (6250 each). Each layer:
  - activations of all nodes are replicated per-core in DRAM (bf16, scaled h/4),
    via a 2-bank split AllGather (banks fire as their tiles finish)
  - each core gathers edge-source rows (bf16, 1KB elems — bandwidth-bound) for
    edges whose dst it owns (dma_gather, <=6-chunk calls: HW desc-ring cap),
    scatter-adds them per 128-dst tile with one-hot matmuls into PSUM
  - the self term x_i is fused into the PSUM->SBUF drain as a DVE add
  - the MLP runs in fp8 e4m3 with DoubleRow pairing (0.5 cyc/row): activations
    are stored as h/4 everywhere (exact bf16 exponent shift), W1 is scaled by
    4 and W2 by 1/4 on the host, so all fp8 tensors stay well inside e4m3
    range and the arithmetic is exact up to fp8 rounding
  - layer 1 aggregation also runs fp8 DoubleRow from host-pre-gathered x/4
Pooling: per-core partial graph sums via one-hot matmul, AllReduce, then
counts/fc/sigmoid replicated on every core. Spectral norm of the weights and
all edge bucketing run on the host in numpy.
"""

import numpy as np
import ml_dtypes

import concourse.bass as bass
import concourse.bacc as bacc
import concourse.mybir as mybir
import concourse.tile as tile
from concourse.bass_utils import run_bass_kernel_spmd

BF16 = mybir.dt.bfloat16
F32 = mybir.dt.float32
F8 = mybir.dt.float8e4
I16 = mybir.dt.int16
nbf16 = ml_dtypes.bfloat16
nf8 = ml_dtypes.float8_e4m3

# ---------------- problem config (hardcoded for the graded problem) ----------
CORES = 8
N = 50000
E = 800000
G = 64
D_IN = 128
H = 512
N_LAYERS = 4
SN_ITERS = 5

P = 128          # partitions

import os as _os

G4 = int(_os.environ.get("KBASS_G4", "4"))          # tiles per compute group
MAX_GATHER_CHUNKS = int(_os.environ.get("KBASS_MAXCH", "6"))  # HW ring cap ~1008 idx/call
N_SWDGE_QUEUES = int(_os.environ.get("KBASS_NSWQ", "4"))
SWDGE_SCRATCH = int(_os.environ.get("KBASS_SCRATCH", "16384"))
EDGE_BUFS = int(_os.environ.get("KBASS_EBUFS", "7"))
FP8MLP = _os.environ.get("KBASS_FP8MLP", "0") == "1"
FP8L1 = _os.environ.get("KBASS_FP8L1", "1") == "1"


def cdiv(a, b):
    return -(-a // b)


def _bank_geometry(npc, tiles):
    """Tile-aligned bank splits (per-rank row ranges) for the split AllGather.

    Four banks, each AllGathered as soon as its tiles finish, so only the
    last ~quarter-AG is exposed at a layer boundary."""
    nb = min(4, tiles)
    tsplits = sorted({cdiv(tiles * (i + 1), nb) for i in range(nb)})
    starts = [0] + [min(t * P, npc) for t in tsplits]
    return [(starts[i], starts[i + 1]) for i in range(len(tsplits))]


NPC = N // CORES                      # nodes per core
TILES = -(-NPC // P)                  # dst tiles per core
LAST_ROWS = NPC - (TILES - 1) * P     # rows in the last tile
BANKS = _bank_geometry(NPC, TILES)    # [(row_start, row_end) per rank]
NBANKS = len(BANKS)


def groups_list():
    return [list(range(g, min(g + G4, TILES))) for g in range(0, TILES, G4)]


def _no_cc():
    return _os.environ.get("KBASS_NO_CC", "0") == "1"


def _patch_tile_swdge_lanes():
    """Partition Tile's 8 DMASW completion-sem lanes by SWDGE queue instead of
    global round-robin (the default can put DMAs from different queues on one
    lane, breaking the per-lane FIFO-completion invariant Tile assumes)."""
    import concourse.tile_sem_assignment as tsa
    from concourse.tile_scheduler import DMAInst

    if getattr(tsa.TileClockTick, "_kbass_qaware", False):
        return
    orig = tsa.TileClockTick._assign_tick

    def _assign_tick(self, inst):
        if (
            isinstance(inst, DMAInst)
            and inst.engine == mybir.EngineType.Pool
            and not isinstance(inst, bass_isa.UserSyncedRemoteDMADescs)
        ):
            q = getattr(inst, "queue_num", 0) or 0
            lanes_per_q = max(1, self.swdge_sem_count // N_SWDGE_QUEUES)
            if not hasattr(self, "_kbass_qtog"):
                self._kbass_qtog = {}
            tog = self._kbass_qtog.get(q, 0)
            self._kbass_qtog[q] = (tog + 1) % lanes_per_q
            self.next_sw_dma_idx = (q * lanes_per_q + tog) % self.swdge_sem_count
        return orig(self, inst)

    tsa.TileClockTick._assign_tick = _assign_tick
    tsa.TileClockTick._kbass_qaware = True


def configure(n=50000, e=800000, g=64, d_in=128, h=512, n_layers=4):
    """Reconfigure module geometry (used by test harnesses for small smoke runs)."""
    global N, E, G, D_IN, H, N_LAYERS, NPC, TILES, LAST_ROWS, BANKS, NBANKS
    N, E, G, D_IN, H, N_LAYERS = n, e, g, d_in, h, n_layers
    NPC = N // CORES
    TILES = -(-NPC // P)
    LAST_ROWS = NPC - (TILES - 1) * P
    BANKS = _bank_geometry(NPC, TILES)
    NBANKS = len(BANKS)
    _prog_cache.clear()


def tile_rows(t):
    return LAST_ROWS if t == TILES - 1 else P


# ---------------- host-side math ---------------------------------------------
def _spectral_normalize(W):
    W = np.asarray(W, np.float32)
    u = np.ones((W.shape[0],), np.float32) / np.float32(np.sqrt(np.float32(W.shape[0])))
    for _ in range(SN_ITERS):
        v = W.T @ u
        v = v / (np.linalg.norm(v) + np.float32(1e-12))
        u = W @ v
        u = u / (np.linalg.norm(u) + np.float32(1e-12))
    sigma = u @ (W @ v)
    return (W / sigma).astype(np.float32)


def _pack_call(idx, n_chunks):
    """int16 idxs for one dma_gather call: index i lives at [i%16, i//16],
    replicated across the eight 16-partition groups (one per Q7 core)."""
    L = np.zeros((n_chunks * P,), np.int16)
    L[: len(idx)] = idx.astype(np.int16)
    return np.tile(L.reshape(-1, 16).T, (8, 1))  # [128, n_chunks*8]


def _preprocess_edges(edge_index, x0q):
    """Bucket edges by (dst core, dst tile, src bank); uniform chunk counts.

    Returns:
      nch    [TILES, NBANKS] per-(tile,bank) 128-edge chunk counts (max/cores)
      l1ch   [TILES] per-tile chunk count rounded up to even (layer-1 DR pairs;
             the pad chunk has an all-zero one-hot)
      idx16  [CORES, 128, tot_ch*8] gather idx packed per (t, b, <=6ch piece)
      dlocs  [CORES, 128, l1tot] bf16 dst slots (-1 pads) in tile-major order
      x1g    [CORES, 128, l1tot*128] fp8 layer-1 pre-gathered x/4 edge feats
    """
    src = np.asarray(edge_index[0], np.int64)
    dst = np.asarray(edge_index[1], np.int64)
    core = dst // NPC
    tloc = (dst % NPC) // P
    dloc = (dst % NPC) % P
    r = src // NPC
    i = src % NPC
    bstarts = np.array([b[0] for b in BANKS] + [NPC], np.int64)
    bank = np.searchsorted(bstarts, i, side="right") - 1
    brows = bstarts[1:] - bstarts[:-1]
    srcloc = r * brows[bank] + (i - bstarts[bank])

    key = (core * TILES + tloc) * NBANKS + bank
    # secondary sort by srcloc: ascending gather addresses within each bucket
    # (better HBM locality for the 1KB random reads)
    order = np.lexsort((srcloc, key))
    key_s, srcloc_s, dloc_s, src_s = key[order], srcloc[order], dloc[order], src[order]
    counts = np.bincount(key_s, minlength=CORES * TILES * NBANKS).reshape(
        CORES, TILES, NBANKS
    )
    starts = np.zeros(CORES * TILES * NBANKS + 1, np.int64)
    np.cumsum(counts.reshape(-1), out=starts[1:])

    nch = np.maximum(cdiv(counts.max(axis=0), P), 1)  # [TILES, NBANKS]
    ncht = nch.sum(axis=1)                            # [TILES]
    tot_ch = int(ncht.sum())
    l1ch = ncht + (ncht % 2)                          # even for L1 DR pairs
    l1off = np.zeros(TILES + 1, np.int64)
    np.cumsum(l1ch, out=l1off[1:])
    l1tot = int(l1off[-1])

    idx16 = np.zeros((CORES, P, tot_ch * 8), np.int16)
    dlocs = np.full((CORES, P, l1tot), -1.0, nbf16)
    x1g = np.zeros((CORES, P, l1tot * P), nf8)
    for c in range(CORES):
        icol = 0
        for t in range(TILES):
            for b in range(NBANKS):
                k = (c * TILES + t) * NBANKS + b
                s, e = starts[k], starts[k + 1]
                nchb = int(nch[t, b])
                bidx = np.zeros((nchb * P,), np.int64)
                bidx[: e - s] = srcloc_s[s:e]
                done = 0
                while done < nchb:
                    npiece = min(MAX_GATHER_CHUNKS, nchb - done)
                    idx16[c, :, icol : icol + npiece * 8] = _pack_call(
                        bidx[done * P : (done + npiece) * P], npiece
                    )
                    icol += npiece * 8
                    done += npiece
        for t in range(TILES):
            dcol = int(l1off[t])
            for b in range(NBANKS):
                k = (c * TILES + t) * NBANKS + b
                s, e = starts[k], starts[k + 1]
                nchb = int(nch[t, b])
                dl = np.full((nchb * P,), -1.0, np.float32)
                dl[: e - s] = dloc_s[s:e]
                dlocs[c, :, dcol : dcol + nchb] = dl.reshape(nchb, P).T.astype(nbf16)
                gsrc = np.zeros((nchb * P,), np.int64)
                gsrc[: e - s] = src_s[s:e]
                x1g[c, :, dcol * P : (dcol + nchb) * P] = (
                    x0q[gsrc]
                    .reshape(nchb, P, D_IN)
                    .transpose(1, 0, 2)
                    .reshape(P, nchb * D_IN)
                )
                dcol += nchb
    return nch, l1ch, idx16, dlocs, x1g


def _build_pool_onehot(batch):
    batch = np.asarray(batch, np.int64)
    pool = np.zeros((CORES, P, TILES * G), np.float32)
    for c in range(CORES):
        b = batch[c * NPC : (c + 1) * NPC]
        for i in range(NPC):
            t, p = i // P, i % P
            pool[c, p, t * G + int(b[i])] = 1.0
    counts = np.bincount(batch, minlength=G).astype(np.float32)
    cinv = (1.0 / np.maximum(counts, 1.0)).astype(np.float32)
    return pool, cinv


# ---------------- device program ---------------------------------------------
from concourse import bass_isa


def build_program(nch, l1ch):
    _patch_tile_swdge_lanes()
    nch = np.asarray(nch)
    l1ch = np.asarray(l1ch)
    ncht = nch.sum(axis=1)
    ncht_max = int(ncht.max())
    l1max = int(l1ch.max())
    l1off = np.zeros(TILES + 1, np.int64)
    np.cumsum(l1ch, out=l1off[1:])
    l1tot = int(l1off[-1])
    grs = groups_list()
    NG = len(grs)
    icol_off = np.zeros((TILES, NBANKS), np.int64)
    acc = 0
    for t in range(TILES):
        for b in range(NBANKS):
            icol_off[t, b] = acc
            acc += int(nch[t, b]) * 8
    idx_cols = acc
    WD = F8 if FP8MLP else BF16  # MLP weight/act dtype
    L1D = F8 if FP8L1 else BF16  # layer-1 edge dtype
    DR = mybir.MatmulPerfMode.DoubleRow

    nc = bacc.Bacc(
        num_devices=CORES,
        target_bir_lowering=False,
        debug=False,
        num_swdge_queues=N_SWDGE_QUEUES,
        dynamic_dma_scratch_size=SWDGE_SCRATCH,
    )

    # ---- external inputs
    x1g = nc.declare_dram_parameter("x1g", [P, l1tot * P], L1D, isOutput=False)
    xown0 = nc.declare_dram_parameter("xown0", [NPC, D_IN], BF16, isOutput=False)
    idx16 = nc.declare_dram_parameter("idx16", [P, idx_cols], I16, isOutput=False)
    dlocs = nc.declare_dram_parameter("dlocs", [P, l1tot], BF16, isOutput=False)
    iotar = nc.declare_dram_parameter("iotar", [P, l1max * P], BF16, isOutput=False)
    pool1h = nc.declare_dram_parameter("pool1h", [P, TILES * G], BF16, isOutput=False)
    w1t0 = nc.declare_dram_parameter("w1t0", [D_IN, H], WD, isOutput=False)
    w1tr = nc.declare_dram_parameter("w1tr", [(N_LAYERS - 1) * H, H], WD, isOutput=False)
    w2t = nc.declare_dram_parameter("w2t", [N_LAYERS * H, H], WD, isOutput=False)
    b1c = nc.declare_dram_parameter("b1c", [P, N_LAYERS * 4], F32, isOutput=False)
    b2bc = nc.declare_dram_parameter("b2bc", [N_LAYERS * P, H], F32, isOutput=False)
    ident16 = nc.declare_dram_parameter("ident16", [P, P], BF16, isOutput=False)
    cinv = nc.declare_dram_parameter("cinv", [G, 1], F32, isOutput=False)
    fcwb = nc.declare_dram_parameter("fcwb", [G, H], F32, isOutput=False)
    fcb = nc.declare_dram_parameter("fcb", [G, 1], F32, isOutput=False)
    out_ext = nc.declare_dram_parameter("out", [G, 1], F32, isOutput=True)

    # ---- internal DRAM (double-buffered per layer parity)
    agx = [
        [
            nc.dram_tensor(f"ag{b}_{i}", [BANKS[b][1] - BANKS[b][0], H], BF16)
            for b in range(NBANKS)
        ]
        for i in range(2)
    ]
    xfx = [
        [
            nc.dram_tensor(
                f"xf{b}_{i}",
                [CORES * (BANKS[b][1] - BANKS[b][0]), H],
                BF16,
                addr_space="Shared",
            )
            for b in range(NBANKS)
        ]
        for i in range(2)
    ]
    prb = nc.dram_tensor("prb", [G, H], F32)
    pro = nc.dram_tensor("pro", [G, H], F32, addr_space="Shared")

    rg = [list(range(CORES))]

    with tile.TileContext(nc) as tc:
        with (
            tc.tile_pool(name="consts", bufs=1) as cpool,
            tc.tile_pool(name="wts", bufs=2) as wpool,
            tc.tile_pool(name="edge", bufs=EDGE_BUFS) as epool,
            tc.tile_pool(name="bsel", bufs=7) as bpool,
            tc.tile_pool(name="xo", bufs=2 * G4) as xopool,
            tc.tile_pool(name="hsb", bufs=5) as hpool,
            tc.tile_pool(name="hfm", bufs=2) as fpool,
            tc.tile_pool(name="zt", bufs=2) as zpool,
            tc.tile_pool(name="agt", bufs=4) as agpool,
            tc.tile_pool(name="ps_agg", bufs=2, space="PSUM") as agg_ps,
            tc.tile_pool(name="ps_tp", bufs=1, space="PSUM") as tp_ps,
            tc.tile_pool(name="ps_z", bufs=2, space="PSUM") as z_ps,
            tc.tile_pool(name="ps_h2", bufs=2, space="PSUM") as h2_ps,
            tc.tile_pool(name="ps_pool", bufs=1, space="PSUM") as pool_ps,
        ):
            # ---- load constants
            idx_sb = cpool.tile([P, idx_cols], I16)
            nc.sync.dma_start(idx_sb[:], idx16[:, :])
            dloc_sb = cpool.tile([P, l1tot], BF16)
            nc.sync.dma_start(dloc_sb[:], dlocs[:, :])
            iota_sb = cpool.tile([P, l1max * P], BF16)
            nc.sync.dma_start(iota_sb[:], iotar[:, :])
            id16_sb = cpool.tile([P, P], BF16)
            nc.sync.dma_start(id16_sb[:], ident16[:, :])
            b1_sb = cpool.tile([P, N_LAYERS * 4], F32)
            nc.sync.dma_start(b1_sb[:], b1c[:, :])
            cinv_sb = cpool.tile([G, 1], F32)
            nc.sync.dma_start(cinv_sb[:], cinv[:, :])
            fcw_sb = cpool.tile([G, H], F32)
            nc.sync.dma_start(fcw_sb[:], fcwb[:, :])
            fcb_sb = cpool.tile([G, 1], F32)
            nc.sync.dma_start(fcb_sb[:], fcb[:, :])
            pool_sb = cpool.tile([P, TILES * G], BF16)
            nc.sync.dma_start(pool_sb[:], pool1h[:, :])

            self_qn = [0]  # rotating SWDGE queue assignment for gathers

            def issue_gathers(lay, t):
                """Per-(tile, bank) dma_gather pieces (<= MAX_GATHER_CHUNKS).

                Returns [(et_tile, n_chunks)] in tile-major chunk order."""
                banks_src = [t_[:, :] for t_ in xfx[(lay - 1) % 2]]
                calls = []
                for b in range(NBANKS):
                    nchb = int(nch[t, b])
                    icol = int(icol_off[t, b])
                    done = 0
                    while done < nchb:
                        npiece = min(MAX_GATHER_CHUNKS, nchb - done)
                        et = epool.tile(
                            [P, MAX_GATHER_CHUNKS * H], BF16, tag="etile"
                        )
                        nc.gpsimd.dma_gather(
                            out_ap=et[:, 0 : npiece * H].rearrange(
                                "p (s e) -> p s e", e=H
                            ),
                            in_ap=banks_src[b],
                            idxs_ap=idx_sb[:, icol : icol + npiece * 8],
                            num_idxs=npiece * P,
                            num_idxs_reg=npiece * P,
                            elem_size=H,
                            queue_num=self_qn[0] % N_SWDGE_QUEUES,
                        )
                        self_qn[0] += 1
                        calls.append((et, npiece))
                        icol += npiece * 8
                        done += npiece
                return calls

            def load_pre(lay, t):
                """bsel one-hot (DVE is_equal) + self-term xo for tile t."""
                rows = tile_rows(t)
                nb = int(l1ch[t]) if lay == 0 else int(ncht[t])
                dcol = int(l1off[t])
                if lay == 0:
                    bs = bpool.tile([P, l1max * P], L1D, tag="bs0")
                else:
                    bs = bpool.tile([P, l1max * P], BF16, tag="bs")
                nc.vector.tensor_tensor(
                    out=bs[:, 0 : nb * P].rearrange("p (s j) -> p s j", j=P),
                    in0=iota_sb[:, 0 : nb * P].rearrange("p (s j) -> p s j", j=P),
                    in1=dloc_sb[:, dcol : dcol + nb, None].broadcast_to([P, nb, P]),
                    op=mybir.AluOpType.is_equal,
                )
                if lay == 0:
                    xo = xopool.tile([P, D_IN], BF16, tag="xo0")
                    if rows < P:
                        nc.vector.memset(xo[:], 0.0)
                    nc.sync.dma_start(xo[:rows, :], xown0[t * P : t * P + rows, :])
                else:
                    xo = xopool.tile([P, H], BF16, tag="xo")
                    if rows < P:
                        nc.vector.memset(xo[:], 0.0)
                    bt = next(
                        bi for bi, (s0, e0) in enumerate(BANKS) if s0 <= t * P < e0
                    )
                    o = t * P - BANKS[bt][0]
                    nc.sync.dma_start(
                        xo[:rows, :], agx[(lay - 1) % 2][bt][o : o + rows, :]
                    )
                return bs, xo

            for lay in range(N_LAYERS):
                din = D_IN if lay == 0 else H
                fch = din // P  # feature chunks of the layer input

                # per-layer weights (bufs=2 -> prefetch overlaps prev layer)
                w1t_sb = wpool.tile([P, 4 * H], WD, tag="w1t")
                if lay == 0:
                    nc.sync.dma_start(w1t_sb[:, 0:H], w1t0[:, :])
                else:
                    for fi in range(fch):
                        nc.sync.dma_start(
                            w1t_sb[:, fi * H : (fi + 1) * H],
                            w1tr[(lay - 1) * H + fi * P : (lay - 1) * H + (fi + 1) * P, :],
                        )
                w2t_sb = wpool.tile([P, 4 * H], WD, tag="w2t")
                for zf in range(4):
                    nc.sync.dma_start(
                        w2t_sb[:, zf * H : (zf + 1) * H],
                        w2t[lay * H + zf * P : lay * H + (zf + 1) * P, :],
                    )
                b2_sb = wpool.tile([P, H], F32, tag="b2")
                nc.sync.dma_start(b2_sb[:], b2bc[lay * P : (lay + 1) * P, :])

                if lay == N_LAYERS - 1:
                    poolps = pool_ps.tile([G, H], F32)

                def issue_edges(t):
                    if lay > 0:
                        return issue_gathers(lay, t)
                    nb = int(l1ch[t])
                    et = epool.tile([P, l1max * D_IN], L1D, tag="etL1")
                    nc.sync.dma_start(
                        et[:, 0 : nb * D_IN],
                        x1g[:, int(l1off[t]) * P : (int(l1off[t]) + nb) * P],
                    )
                    return [(et, nb)]

                # prime group 0's edge data + bsel/xo
                calls_cur = {t: issue_edges(t) for t in grs[0]}
                pre_cur = {t: load_pre(lay, t) for t in grs[0]}

                for gi, gts in enumerate(grs):
                    calls_g = calls_cur
                    pre = pre_cur
                    if gi + 1 < NG:
                        calls_cur = {t: issue_edges(t) for t in grs[gi + 1]}
                        pre_cur = {t: load_pre(lay, t) for t in grs[gi + 1]}

                    nodes_c = sum(tile_rows(t) for t in gts)

                    # -- scatter-add matmuls per tile
                    h_tiles = []
                    for t in gts:
                        bs, xo = pre[t]
                        aggps = agg_ps.tile([P, din], F32, tag="agg")
                        if lay == 0:
                            nb = int(l1ch[t])
                            bsr = bs[:, 0 : nb * P].rearrange("p (s j) -> p s j", j=P)
                            et, _ = calls_g[t][0]
                            etr = et[:, 0 : nb * D_IN].rearrange(
                                "p (s e) -> p s e", e=D_IN
                            )
                            if FP8L1:
                                for k in range(nb // 2):
                                    nc.tensor.matmul(
                                        aggps[:],
                                        lhsT=bsr[:, 2 * k : 2 * k + 2, :],
                                        rhs=etr[:, 2 * k : 2 * k + 2, :],
                                        start=(k == 0),
                                        stop=(k == nb // 2 - 1),
                                        perf_mode=DR,
                                    )
                            else:
                                for k in range(nb):
                                    nc.tensor.matmul(
                                        aggps[:],
                                        lhsT=bsr[:, k, :],
                                        rhs=etr[:, k, :],
                                        start=(k == 0),
                                        stop=(k == nb - 1),
                                    )
                        else:
                            ncht_t = int(ncht[t])
                            bsr = bs[:, 0 : ncht_t * P].rearrange(
                                "p (s j) -> p s j", j=P
                            )
                            k = 0
                            for et, npiece in calls_g[t]:
                                etr = et[:, 0 : npiece * H].rearrange(
                                    "p (s e) -> p s e", e=H
                                )
                                for j in range(npiece):
                                    nc.tensor.matmul(
                                        aggps[:],
                                        lhsT=bsr[:, k, :],
                                        rhs=etr[:, j, :],
                                        start=(k == 0),
                                        stop=(k == ncht_t - 1),
                                    )
                                    k += 1
                        # self term fused into the PSUM drain (h/4 domain)
                        h_sb = hpool.tile([P, din], BF16, tag="h")
                        nc.vector.tensor_tensor(
                            out=h_sb[:],
                            in0=aggps[:],
                            in1=xo[:],
                            op=mybir.AluOpType.add,
                        )
                        h_tiles.append(h_sb)

                    # transpose h -> feature-major, then cast to the MLP dtype
                    hfm = fpool.tile([P, fch * 512], BF16, tag="hfm")
                    for ti, t in enumerate(gts):
                        tps = tp_ps.tile([P, fch * P], BF16, tag="tp")
                        for f in range(fch):
                            nc.tensor.transpose(
                                out=tps[:, f * P : (f + 1) * P],
                                in_=h_tiles[ti][:, f * P : (f + 1) * P],
                                identity=id16_sb[:],
                            )
                        nc.vector.tensor_copy(
                            hfm[:, 0 : fch * 512].rearrange("p (f n) -> p f n", n=512)[
                                :, :, ti * P : (ti + 1) * P
                            ],
                            tps[:, 0 : fch * P].rearrange("p (f j) -> p f j", j=P),
                        )
                    if FP8MLP:
                        hfm8 = fpool.tile([P, fch * 512], F8, tag="hfm8")
                        nc.vector.tensor_copy(
                            hfm8[:, 0 : fch * 512].rearrange(
                                "p (f n) -> p f n", n=512
                            )[:, :, 0:nodes_c],
                            hfm[:, 0 : fch * 512].rearrange(
                                "p (f n) -> p f n", n=512
                            )[:, :, 0:nodes_c],
                        )
                        hin = hfm8
                    else:
                        hin = hfm
                    hinr = hin[:, 0 : fch * 512].rearrange("p (f n) -> p f n", n=512)
                    w1r = w1t_sb[:, 0 : fch * H].rearrange("p (f o) -> p f o", o=H)

                    # MLP1: z = relu(h @ W1T + b1), feature-major
                    z8 = zpool.tile([P, 4 * 512], WD, tag="z8")
                    z8r = z8[:, 0 : 4 * 512].rearrange("p (f n) -> p f n", n=512)
                    for fo in range(4):
                        zps = z_ps.tile([P, 512], F32, tag="z")
                        if FP8MLP and fch > 1:
                            for m in range(fch // 2):
                                nc.tensor.matmul(
                                    zps[:, :nodes_c],
                                    lhsT=w1r[:, 2 * m : 2 * m + 2, fo * P : (fo + 1) * P],
                                    rhs=hinr[:, 2 * m : 2 * m + 2, 0:nodes_c],
                                    start=(m == 0),
                                    stop=(m == fch // 2 - 1),
                                    perf_mode=DR,
                                )
                        else:
                            for fi in range(fch):
                                nc.tensor.matmul(
                                    zps[:, :nodes_c],
                                    lhsT=w1r[:, fi, fo * P : (fo + 1) * P],
                                    rhs=hinr[:, fi, 0:nodes_c],
                                    start=(fi == 0),
                                    stop=(fi == fch - 1),
                                )
                        nc.scalar.activation(
                            z8r[:, fo, 0:nodes_c],
                            zps[:, :nodes_c],
                            mybir.ActivationFunctionType.Relu,
                            bias=b1_sb[:, lay * 4 + fo : lay * 4 + fo + 1],
                        )

                    # MLP2: h_next = z @ W2T + b2, node-major
                    w2r = w2t_sb[:, 0 : 4 * H].rearrange("p (f o) -> p f o", o=H)
                    for ti, t in enumerate(gts):
                        rows = tile_rows(t)
                        h2ps = h2_ps.tile([P, H], F32, tag="h2")
                        if FP8MLP:
                            for m in range(2):
                                nc.tensor.matmul(
                                    h2ps[:rows, :],
                                    lhsT=z8r[:, 2 * m : 2 * m + 2, ti * P : ti * P + rows],
                                    rhs=w2r[:, 2 * m : 2 * m + 2, :],
                                    start=(m == 0),
                                    stop=(m == 1),
                                    perf_mode=DR,
                                )
                        else:
                            for zf in range(4):
                                nc.tensor.matmul(
                                    h2ps[:rows, :],
                                    lhsT=z8r[:, zf, ti * P : ti * P + rows],
                                    rhs=w2r[:, zf, :],
                                    start=(zf == 0),
                                    stop=(zf == 3),
                                )
                        if lay < N_LAYERS - 1:
                            agt = agpool.tile([P, H], BF16, tag="ag")
                            nc.vector.tensor_tensor(
                                out=agt[:rows, :],
                                in0=h2ps[:rows, :],
                                in1=b2_sb[:rows, :],
                                op=mybir.AluOpType.add,
                            )
                            bt = next(
                                bi for bi, (s0, e0) in enumerate(BANKS)
                                if s0 <= t * P < e0
                            )
                            o = t * P - BANKS[bt][0]
                            nc.sync.dma_start(
                                agx[lay % 2][bt][o : o + rows, :], agt[:rows, :]
                            )
                        else:
                            hn = agpool.tile([P, H], BF16, tag="hn")
                            nc.vector.tensor_tensor(
                                out=hn[:rows, :],
                                in0=h2ps[:rows, :],
                                in1=b2_sb[:rows, :],
                                op=mybir.AluOpType.add,
                            )
                            nc.tensor.matmul(
                                poolps[:],
                                lhsT=pool_sb[:rows, t * G : (t + 1) * G],
                                rhs=hn[:rows, :],
                                start=(t == 0),
                                stop=(t == TILES - 1),
                            )

                    # split AllGather: each bank fires as soon as its tiles are done
                    if lay < N_LAYERS - 1:
                        for b in range(NBANKS):
                            bank_done = cdiv(BANKS[b][1], P) - 1
                            if bank_done not in gts:
                                continue
                            agt_, xft_ = agx[lay % 2][b], xfx[lay % 2][b]
                            if _no_cc():
                                nc.sync.dma_start(
                                    xft_[0 : agt_.shape[0], :], agt_[:, :]
                                )
                            else:
                                nc.gpsimd.collective_compute(
                                    "AllGather",
                                    mybir.AluOpType.bypass,
                                    replica_groups=rg,
                                    ins=[agt_[:, :]],
                                    outs=[xft_[:, :]],
                                )

            # ---- pooled epilogue (replicated on every core)
            poolsb = cpool.tile([G, H], F32)
            nc.vector.tensor_copy(poolsb[:], poolps[:])
            nc.sync.dma_start(prb[:, :], poolsb[:])
            if _no_cc():
                nc.sync.dma_start(pro[:, :], prb[:, :])
            else:
                nc.gpsimd.collective_compute(
                    "AllReduce",
                    mybir.AluOpType.add,
                    replica_groups=rg,
                    ins=[prb[:, :]],
                    outs=[pro[:, :]],
                )
            pr_sb = cpool.tile([G, H], F32)
            nc.sync.dma_start(pr_sb[:], pro[:, :])
            nc.vector.tensor_scalar_mul(pr_sb[:], pr_sb[:], cinv_sb[:, 0:1])
            tmp = cpool.tile([G, H], F32)
            nc.vector.tensor_tensor(
                out=tmp[:], in0=pr_sb[:], in1=fcw_sb[:], op=mybir.AluOpType.mult
            )
            dot = cpool.tile([G, 1], F32)
            nc.vector.tensor_reduce(
                out=dot[:], in_=tmp[:], axis=mybir.AxisListType.X, op=mybir.AluOpType.add
            )
            osb = cpool.tile([G, 1], F32)
            nc.scalar.activation(
                osb[:],
                dot[:],
                mybir.ActivationFunctionType.Sigmoid,
                bias=fcb_sb[:, 0:1],
            )
            nc.sync.dma_start(out_ext[:, :], osb[:])

    nc.compile()
    return nc


# ---------------- host wrapper ------------------------------------------------
def _prepare_inputs(x, edge_index, batch, w1_0, b1_0, w2_0, b2_0,
                    w1_rest, b1_rest, w2_rest, b2_rest, fc_w, fc_b):
    # activations live in the h/4 domain on device (exact bf16 exponent shift);
    # W1 is scaled by 4, W2 by 1/4 (except the last layer, which feeds the
    # pool at true scale), so all MLP tensors stay inside fp8 e4m3 range.
    S = np.float32(4.0)
    x0q = np.asarray(x, np.float32) / S
    nch, l1ch, idx16, dlocs, x1g = _preprocess_edges(np.asarray(edge_index), x0q)
    pool, cinv = _build_pool_onehot(batch)
    l1max = int(np.asarray(l1ch).max())

    nwd = nf8 if FP8MLP else nbf16
    w1tl = [_spectral_normalize(w1_0).T * S]
    w2tl = [_spectral_normalize(w2_0).T / S]
    b1l = [np.asarray(b1_0, np.float32)]
    b2l = [np.asarray(b2_0, np.float32) / S]
    for i in range(N_LAYERS - 1):
        last = i == N_LAYERS - 2
        w1tl.append(_spectral_normalize(w1_rest[i]).T * S)
        w2tl.append(_spectral_normalize(w2_rest[i]).T * (np.float32(1.0) if last else 1.0 / S))
        b1l.append(np.asarray(b1_rest[i], np.float32))
        b2l.append(np.asarray(b2_rest[i], np.float32) * (np.float32(1.0) if last else 1.0 / S))

    w1t0_np = np.ascontiguousarray(w1tl[0])                      # [128, 512]
    w1tr_np = np.ascontiguousarray(np.concatenate(w1tl[1:], 0))  # [3*512, 512]
    w2t_np = np.ascontiguousarray(np.concatenate(w2tl, 0))       # [4*512, 512]
    b1c_np = np.zeros((P, N_LAYERS * 4), np.float32)
    for l in range(N_LAYERS):
        for f in range(4):
            b1c_np[:, l * 4 + f] = b1l[l][f * P : (f + 1) * P]
    b2bc_np = np.zeros((N_LAYERS * P, H), np.float32)
    for l in range(N_LAYERS):
        b2bc_np[l * P : (l + 1) * P, :] = b2l[l][None, :]

    iota_np = np.tile(np.arange(P, dtype=np.float32), l1max)[None, :].repeat(P, 0)
    shared = {
        "w1t0": w1t0_np.astype(nwd),
        "w1tr": w1tr_np.astype(nwd),
        "w2t": w2t_np.astype(nwd),
        "b1c": b1c_np,
        "b2bc": b2bc_np,
        "iotar": iota_np.astype(nbf16),
        "ident16": np.eye(P, dtype=np.float32).astype(nbf16),
        "cinv": cinv[:, None],
        "fcwb": np.repeat(np.asarray(fc_w, np.float32), G, axis=0),
        "fcb": np.full((G, 1), np.float32(np.asarray(fc_b).reshape(-1)[0]), np.float32),
    }
    x0q16 = x0q.astype(nbf16)
    in_maps = []
    for c in range(CORES):
        m = dict(shared)
        m["xown0"] = np.ascontiguousarray(x0q16[c * NPC : (c + 1) * NPC])
        m["x1g"] = np.ascontiguousarray(x1g[c]) if FP8L1 else np.ascontiguousarray(
            x1g[c]).astype(nbf16)
        m["idx16"] = np.ascontiguousarray(idx16[c])
        m["dlocs"] = np.ascontiguousarray(dlocs[c])
        m["pool1h"] = np.ascontiguousarray(pool[c]).astype(nbf16)
        in_maps.append(m)
    return nch, l1ch, in_maps


_prog_cache = {}
last_results = None


def kernel(x, edge_index, batch, w1_0, b1_0, w2_0, b2_0,
           w1_rest, b1_rest, w2_rest, b2_rest, fc_w, fc_b, **run_kwargs):
    global last_results
    nch, l1ch, in_maps = _prepare_inputs(
        x, edge_index, batch, w1_0, b1_0, w2_0, b2_0,
        w1_rest, b1_rest, w2_rest, b2_rest, fc_w, fc_b,
    )
    key = np.asarray(nch).tobytes() + np.asarray(l1ch).tobytes()
    if key not in _prog_cache:
        _prog_cache[key] = build_program(nch, l1ch)
    nc = _prog_cache[key]
    res = run_bass_kernel_spmd(nc, in_maps, core_ids=list(range(CORES)), **run_kwargs)
    last_results = res
    return np.asarray(res.results[0]["out"], np.float32)


# revision 24
# speedup vs baseline: 1.4178x; 1.4178x over previous
"""GIN discriminator (4-layer GINConv + global mean pool + sigmoid) on 8 trn2 cores.

Sharding: nodes are split contiguously across 8 cores (6250 each). Each layer:
  - activations of all nodes are replicated per-core in DRAM (bf16, scaled h/4),
    via a 2-bank split AllGather (banks fire as their tiles finish)
  - each core gathers edge-source rows (bf16, 1KB elems — bandwidth-bound) for
    edges whose dst it owns (dma_gather, <=6-chunk calls: HW desc-ring cap),
    scatter-adds them per 128-dst tile with one-hot matmuls into PSUM
  - the self term x_i is fused into the PSUM->SBUF drain as a DVE add
  - the MLP runs in fp8 e4m3 with DoubleRow pairing (0.5 cyc/row): activations
    are stored as h/4 everywhere (exact bf16 exponent shift), W1 is scaled by
    4 and W2 by 1/4 on the host, so all fp8 tensors stay well inside e4m3
    range and the arithmetic is exact up to fp8 rounding
  - layer 1 aggregation also runs fp8 DoubleRow from host-pre-gathered x/4
Pooling: per-core partial graph sums via one-hot matmul, AllReduce, then
counts/fc/sigmoid replicated on every core. Spectral norm of the weights and
all edge bucketing run on the host in numpy.
"""

import numpy as np
import ml_dtypes

import concourse.bass as bass
import concourse.bacc as bacc
import concourse.mybir as mybir
import concourse.tile as tile
from concourse.bass_utils import run_bass_kernel_spmd

BF16 = mybir.dt.bfloat16
F32 = mybir.dt.float32
F8 = mybir.dt.float8e4
I16 = mybir.dt.int16
nbf16 = ml_dtypes.bfloat16
nf8 = ml_dtypes.float8_e4m3

# ---------------- problem config (hardcoded for the graded problem) ----------
CORES = 8
N = 50000
E = 800000
G = 64
D_IN = 128
H = 512
N_LAYERS = 4
SN_ITERS = 5

P = 128          # partitions

import os as _os

G4 = int(_os.environ.get("KBASS_G4", "4"))          # tiles per compute group
MAX_GATHER_CHUNKS = int(_os.environ.get("KBASS_MAXCH", "6"))  # HW ring cap ~1008 idx/call
N_SWDGE_QUEUES = int(_os.environ.get("KBASS_NSWQ", "4"))
SWDGE_SCRATCH = int(_os.environ.get("KBASS_SCRATCH", "16384"))
EDGE_BUFS = int(_os.environ.get("KBASS_EBUFS", "8"))
FP8MLP = _os.environ.get("KBASS_FP8MLP", "1") == "1"
FP8L1 = _os.environ.get("KBASS_FP8L1", "1") == "1"


def _bank_geometry(npc, tiles):
    """Tile-aligned bank splits (per-rank row ranges) for the split AllGather."""
    if tiles >= 2:
        tsplits = [(tiles + 1) // 2, tiles]
    else:
        tsplits = [tiles]
    starts = [0] + [min(t * P, npc) for t in tsplits]
    return [(starts[i], starts[i + 1]) for i in range(len(tsplits))]


NPC = N // CORES                      # nodes per core
TILES = -(-NPC // P)                  # dst tiles per core
LAST_ROWS = NPC - (TILES - 1) * P     # rows in the last tile
BANKS = _bank_geometry(NPC, TILES)    # [(row_start, row_end) per rank]
NBANKS = len(BANKS)


def cdiv(a, b):
    return -(-a // b)


def groups_list():
    return [list(range(g, min(g + G4, TILES))) for g in range(0, TILES, G4)]


def _no_cc():
    return _os.environ.get("KBASS_NO_CC", "0") == "1"


def _patch_tile_swdge_lanes():
    """Partition Tile's 8 DMASW completion-sem lanes by SWDGE queue instead of
    global round-robin (the default can put DMAs from different queues on one
    lane, breaking the per-lane FIFO-completion invariant Tile assumes)."""
    import concourse.tile_sem_assignment as tsa
    from concourse.tile_scheduler import DMAInst

    if getattr(tsa.TileClockTick, "_kbass_qaware", False):
        return
    orig = tsa.TileClockTick._assign_tick

    def _assign_tick(self, inst):
        if (
            isinstance(inst, DMAInst)
            and inst.engine == mybir.EngineType.Pool
            and not isinstance(inst, bass_isa.UserSyncedRemoteDMADescs)
        ):
            q = getattr(inst, "queue_num", 0) or 0
            lanes_per_q = max(1, self.swdge_sem_count // N_SWDGE_QUEUES)
            if not hasattr(self, "_kbass_qtog"):
                self._kbass_qtog = {}
            tog = self._kbass_qtog.get(q, 0)
            self._kbass_qtog[q] = (tog + 1) % lanes_per_q
            self.next_sw_dma_idx = (q * lanes_per_q + tog) % self.swdge_sem_count
        return orig(self, inst)

    tsa.TileClockTick._assign_tick = _assign_tick
    tsa.TileClockTick._kbass_qaware = True


def configure(n=50000, e=800000, g=64, d_in=128, h=512, n_layers=4):
    """Reconfigure module geometry (used by test harnesses for small smoke runs)."""
    global N, E, G, D_IN, H, N_LAYERS, NPC, TILES, LAST_ROWS, BANKS, NBANKS
    N, E, G, D_IN, H, N_LAYERS = n, e, g, d_in, h, n_layers
    NPC = N // CORES
    TILES = -(-NPC // P)
    LAST_ROWS = NPC - (TILES - 1) * P
    BANKS = _bank_geometry(NPC, TILES)
    NBANKS = len(BANKS)
    _prog_cache.clear()


def tile_rows(t):
    return LAST_ROWS if t == TILES - 1 else P


# ---------------- host-side math ---------------------------------------------
def _spectral_normalize(W):
    W = np.asarray(W, np.float32)
    u = np.ones((W.shape[0],), np.float32) / np.float32(np.sqrt(np.float32(W.shape[0])))
    for _ in range(SN_ITERS):
        v = W.T @ u
        v = v / (np.linalg.norm(v) + np.float32(1e-12))
        u = W @ v
        u = u / (np.linalg.norm(u) + np.float32(1e-12))
    sigma = u @ (W @ v)
    return (W / sigma).astype(np.float32)


def _pack_call(idx, n_chunks):
    """int16 idxs for one dma_gather call: index i lives at [i%16, i//16],
    replicated across the eight 16-partition groups (one per Q7 core)."""
    L = np.zeros((n_chunks * P,), np.int16)
    L[: len(idx)] = idx.astype(np.int16)
    return np.tile(L.reshape(-1, 16).T, (8, 1))  # [128, n_chunks*8]


def _preprocess_edges(edge_index, x0q):
    """Bucket edges by (dst core, dst tile, src bank); uniform chunk counts.

    Returns:
      nch    [TILES, NBANKS] per-(tile,bank) 128-edge chunk counts (max/cores)
      l1ch   [TILES] per-tile chunk count rounded up to even (layer-1 DR pairs;
             the pad chunk has an all-zero one-hot)
      idx16  [CORES, 128, tot_ch*8] gather idx packed per (t, b, <=6ch piece)
      dlocs  [CORES, 128, l1tot] bf16 dst slots (-1 pads) in tile-major order
      x1g    [CORES, 128, l1tot*128] fp8 layer-1 pre-gathered x/4 edge feats
    """
    src = np.asarray(edge_index[0], np.int64)
    dst = np.asarray(edge_index[1], np.int64)
    core = dst // NPC
    tloc = (dst % NPC) // P
    dloc = (dst % NPC) % P
    r = src // NPC
    i = src % NPC
    bstarts = np.array([b[0] for b in BANKS] + [NPC], np.int64)
    bank = np.searchsorted(bstarts, i, side="right") - 1
    brows = bstarts[1:] - bstarts[:-1]
    srcloc = r * brows[bank] + (i - bstarts[bank])

    key = (core * TILES + tloc) * NBANKS + bank
    # secondary sort by srcloc: ascending gather addresses within each bucket
    # (better HBM locality for the 1KB random reads)
    order = np.lexsort((srcloc, key))
    key_s, srcloc_s, dloc_s, src_s = key[order], srcloc[order], dloc[order], src[order]
    counts = np.bincount(key_s, minlength=CORES * TILES * NBANKS).reshape(
        CORES, TILES, NBANKS
    )
    starts = np.zeros(CORES * TILES * NBANKS + 1, np.int64)
    np.cumsum(counts.reshape(-1), out=starts[1:])

    nch = np.maximum(cdiv(counts.max(axis=0), P), 1)  # [TILES, NBANKS]
    ncht = nch.sum(axis=1)                            # [TILES]
    tot_ch = int(ncht.sum())
    l1ch = ncht + (ncht % 2)                          # even for L1 DR pairs
    l1off = np.zeros(TILES + 1, np.int64)
    np.cumsum(l1ch, out=l1off[1:])
    l1tot = int(l1off[-1])

    idx16 = np.zeros((CORES, P, tot_ch * 8), np.int16)
    dlocs = np.full((CORES, P, l1tot), -1.0, nbf16)
    x1g = np.zeros((CORES, P, l1tot * P), nf8)
    for c in range(CORES):
        icol = 0
        for t in range(TILES):
            for b in range(NBANKS):
                k = (c * TILES + t) * NBANKS + b
                s, e = starts[k], starts[k + 1]
                nchb = int(nch[t, b])
                bidx = np.zeros((nchb * P,), np.int64)
                bidx[: e - s] = srcloc_s[s:e]
                done = 0
                while done < nchb:
                    npiece = min(MAX_GATHER_CHUNKS, nchb - done)
                    idx16[c, :, icol : icol + npiece * 8] = _pack_call(
                        bidx[done * P : (done + npiece) * P], npiece
                    )
                    icol += npiece * 8
                    done += npiece
        for t in range(TILES):
            dcol = int(l1off[t])
            for b in range(NBANKS):
                k = (c * TILES + t) * NBANKS + b
                s, e = starts[k], starts[k + 1]
                nchb = int(nch[t, b])
                dl = np.full((nchb * P,), -1.0, np.float32)
                dl[: e - s] = dloc_s[s:e]
                dlocs[c, :, dcol : dcol + nchb] = dl.reshape(nchb, P).T.astype(nbf16)
                gsrc = np.zeros((nchb * P,), np.int64)
                gsrc[: e - s] = src_s[s:e]
                x1g[c, :, dcol * P : (dcol + nchb) * P] = (
                    x0q[gsrc]
                    .reshape(nchb, P, D_IN)
                    .transpose(1, 0, 2)
                    .reshape(P, nchb * D_IN)
                )
                dcol += nchb
    return nch, l1ch, idx16, dlocs, x1g


def _build_pool_onehot(batch):
    batch = np.asarray(batch, np.int64)
    pool = np.zeros((CORES, P, TILES * G), np.float32)
    for c in range(CORES):
        b = batch[c * NPC : (c + 1) * NPC]
        for i in range(NPC):
            t, p = i // P, i % P
            pool[c, p, t * G + int(b[i])] = 1.0
    counts = np.bincount(batch, minlength=G).astype(np.float32)
    cinv = (1.0 / np.maximum(counts, 1.0)).astype(np.float32)
    return pool, cinv


# ---------------- device program ---------------------------------------------
from concourse import bass_isa


def build_program(nch, l1ch):
    _patch_tile_swdge_lanes()
    nch = np.asarray(nch)
    l1ch = np.asarray(l1ch)
    ncht = nch.sum(axis=1)
    ncht_max = int(ncht.max())
    l1max = int(l1ch.max())
    l1off = np.zeros(TILES + 1, np.int64)
    np.cumsum(l1ch, out=l1off[1:])
    l1tot = int(l1off[-1])
    grs = groups_list()
    NG = len(grs)
    icol_off = np.zeros((TILES, NBANKS), np.int64)
    acc = 0
    for t in range(TILES):
        for b in range(NBANKS):
            icol_off[t, b] = acc
            acc += int(nch[t, b]) * 8
    idx_cols = acc
    WD = F8 if FP8MLP else BF16  # MLP weight/act dtype
    L1D = F8 if FP8L1 else BF16  # layer-1 edge dtype
    DR = mybir.MatmulPerfMode.DoubleRow

    nc = bacc.Bacc(
        num_devices=CORES,
        target_bir_lowering=False,
        debug=False,
        num_swdge_queues=N_SWDGE_QUEUES,
        dynamic_dma_scratch_size=SWDGE_SCRATCH,
    )

    # ---- external inputs
    x1g = nc.declare_dram_parameter("x1g", [P, l1tot * P], L1D, isOutput=False)
    xown0 = nc.declare_dram_parameter("xown0", [NPC, D_IN], BF16, isOutput=False)
    idx16 = nc.declare_dram_parameter("idx16", [P, idx_cols], I16, isOutput=False)
    dlocs = nc.declare_dram_parameter("dlocs", [P, l1tot], BF16, isOutput=False)
    iotar = nc.declare_dram_parameter("iotar", [P, l1max * P], BF16, isOutput=False)
    pool1h = nc.declare_dram_parameter("pool1h", [P, TILES * G], BF16, isOutput=False)
    w1t0 = nc.declare_dram_parameter("w1t0", [D_IN, H], WD, isOutput=False)
    w1tr = nc.declare_dram_parameter("w1tr", [(N_LAYERS - 1) * H, H], WD, isOutput=False)
    w2t = nc.declare_dram_parameter("w2t", [N_LAYERS * H, H], WD, isOutput=False)
    b1c = nc.declare_dram_parameter("b1c", [P, N_LAYERS * 4], F32, isOutput=False)
    b2bc = nc.declare_dram_parameter("b2bc", [N_LAYERS * P, H], F32, isOutput=False)
    ident16 = nc.declare_dram_parameter("ident16", [P, P], BF16, isOutput=False)
    cinv = nc.declare_dram_parameter("cinv", [G, 1], F32, isOutput=False)
    fcwb = nc.declare_dram_parameter("fcwb", [G, H], F32, isOutput=False)
    fcb = nc.declare_dram_parameter("fcb", [G, 1], F32, isOutput=False)
    out_ext = nc.declare_dram_parameter("out", [G, 1], F32, isOutput=True)

    # ---- internal DRAM (double-buffered per layer parity)
    agx = [
        [
            nc.dram_tensor(f"ag{b}_{i}", [BANKS[b][1] - BANKS[b][0], H], BF16)
            for b in range(NBANKS)
        ]
        for i in range(2)
    ]
    xfx = [
        [
            nc.dram_tensor(
                f"xf{b}_{i}",
                [CORES * (BANKS[b][1] - BANKS[b][0]), H],
                BF16,
                addr_space="Shared",
            )
            for b in range(NBANKS)
        ]
        for i in range(2)
    ]
    prb = nc.dram_tensor("prb", [G, H], F32)
    pro = nc.dram_tensor("pro", [G, H], F32, addr_space="Shared")

    rg = [list(range(CORES))]

    with tile.TileContext(nc) as tc:
        with (
            tc.tile_pool(name="consts", bufs=1) as cpool,
            tc.tile_pool(name="wts", bufs=2) as wpool,
            tc.tile_pool(name="edge", bufs=EDGE_BUFS) as epool,
            tc.tile_pool(name="bsel", bufs=2 * G4) as bpool,
            tc.tile_pool(name="xo", bufs=2 * G4) as xopool,
            tc.tile_pool(name="hsb", bufs=5) as hpool,
            tc.tile_pool(name="hfm", bufs=2) as fpool,
            tc.tile_pool(name="zt", bufs=3) as zpool,
            tc.tile_pool(name="agt", bufs=4) as agpool,
            tc.tile_pool(name="ps_agg", bufs=2, space="PSUM") as agg_ps,
            tc.tile_pool(name="ps_tp", bufs=1, space="PSUM") as tp_ps,
            tc.tile_pool(name="ps_z", bufs=2, space="PSUM") as z_ps,
            tc.tile_pool(name="ps_h2", bufs=2, space="PSUM") as h2_ps,
            tc.tile_pool(name="ps_pool", bufs=1, space="PSUM") as pool_ps,
        ):
            # ---- load constants
            idx_sb = cpool.tile([P, idx_cols], I16)
            nc.sync.dma_start(idx_sb[:], idx16[:, :])
            dloc_sb = cpool.tile([P, l1tot], BF16)
            nc.sync.dma_start(dloc_sb[:], dlocs[:, :])
            iota_sb = cpool.tile([P, l1max * P], BF16)
            nc.sync.dma_start(iota_sb[:], iotar[:, :])
            id16_sb = cpool.tile([P, P], BF16)
            nc.sync.dma_start(id16_sb[:], ident16[:, :])
            b1_sb = cpool.tile([P, N_LAYERS * 4], F32)
            nc.sync.dma_start(b1_sb[:], b1c[:, :])
            cinv_sb = cpool.tile([G, 1], F32)
            nc.sync.dma_start(cinv_sb[:], cinv[:, :])
            fcw_sb = cpool.tile([G, H], F32)
            nc.sync.dma_start(fcw_sb[:], fcwb[:, :])
            fcb_sb = cpool.tile([G, 1], F32)
            nc.sync.dma_start(fcb_sb[:], fcb[:, :])
            pool_sb = cpool.tile([P, TILES * G], BF16)
            nc.sync.dma_start(pool_sb[:], pool1h[:, :])

            self_qn = [0]  # rotating SWDGE queue assignment for gathers

            def issue_gathers(lay, t):
                """Per-(tile, bank) dma_gather pieces (<= MAX_GATHER_CHUNKS).

                Returns [(et_tile, n_chunks)] in tile-major chunk order."""
                banks_src = [t_[:, :] for t_ in xfx[(lay - 1) % 2]]
                calls = []
                for b in range(NBANKS):
                    nchb = int(nch[t, b])
                    icol = int(icol_off[t, b])
                    done = 0
                    while done < nchb:
                        npiece = min(MAX_GATHER_CHUNKS, nchb - done)
                        et = epool.tile(
                            [P, MAX_GATHER_CHUNKS * H], BF16, tag="etile"
                        )
                        nc.gpsimd.dma_gather(
                            out_ap=et[:, 0 : npiece * H].rearrange(
                                "p (s e) -> p s e", e=H
                            ),
                            in_ap=banks_src[b],
                            idxs_ap=idx_sb[:, icol : icol + npiece * 8],
                            num_idxs=npiece * P,
                            num_idxs_reg=npiece * P,
                            elem_size=H,
                            queue_num=self_qn[0] % N_SWDGE_QUEUES,
                        )
                        self_qn[0] += 1
                        calls.append((et, npiece))
                        icol += npiece * 8
                        done += npiece
                return calls

            def load_pre(lay, t):
                """bsel one-hot (DVE is_equal) + self-term xo for tile t."""
                rows = tile_rows(t)
                nb = int(l1ch[t]) if lay == 0 else int(ncht[t])
                dcol = int(l1off[t])
                if lay == 0:
                    bs = bpool.tile([P, l1max * P], L1D, tag="bs0")
                else:
                    bs = bpool.tile([P, l1max * P], BF16, tag="bs")
                nc.vector.tensor_tensor(
                    out=bs[:, 0 : nb * P].rearrange("p (s j) -> p s j", j=P),
                    in0=iota_sb[:, 0 : nb * P].rearrange("p (s j) -> p s j", j=P),
                    in1=dloc_sb[:, dcol : dcol + nb, None].broadcast_to([P, nb, P]),
                    op=mybir.AluOpType.is_equal,
                )
                if lay == 0:
                    xo = xopool.tile([P, D_IN], BF16, tag="xo0")
                    if rows < P:
                        nc.vector.memset(xo[:], 0.0)
                    nc.sync.dma_start(xo[:rows, :], xown0[t * P : t * P + rows, :])
                else:
                    xo = xopool.tile([P, H], BF16, tag="xo")
                    if rows < P:
                        nc.vector.memset(xo[:], 0.0)
                    bt = next(
                        bi for bi, (s0, e0) in enumerate(BANKS) if s0 <= t * P < e0
                    )
                    o = t * P - BANKS[bt][0]
                    nc.sync.dma_start(
                        xo[:rows, :], agx[(lay - 1) % 2][bt][o : o + rows, :]
                    )
                return bs, xo

            for lay in range(N_LAYERS):
                din = D_IN if lay == 0 else H
                fch = din // P  # feature chunks of the layer input

                # per-layer weights (bufs=2 -> prefetch overlaps prev layer)
                w1t_sb = wpool.tile([P, 4 * H], WD, tag="w1t")
                if lay == 0:
                    nc.sync.dma_start(w1t_sb[:, 0:H], w1t0[:, :])
                else:
                    for fi in range(fch):
                        nc.sync.dma_start(
                            w1t_sb[:, fi * H : (fi + 1) * H],
                            w1tr[(lay - 1) * H + fi * P : (lay - 1) * H + (fi + 1) * P, :],
                        )
                w2t_sb = wpool.tile([P, 4 * H], WD, tag="w2t")
                for zf in range(4):
                    nc.sync.dma_start(
                        w2t_sb[:, zf * H : (zf + 1) * H],
                        w2t[lay * H + zf * P : lay * H + (zf + 1) * P, :],
                    )
                b2_sb = wpool.tile([P, H], F32, tag="b2")
                nc.sync.dma_start(b2_sb[:], b2bc[lay * P : (lay + 1) * P, :])

                if lay == N_LAYERS - 1:
                    poolps = pool_ps.tile([G, H], F32)

                def issue_edges(t):
                    if lay > 0:
                        return issue_gathers(lay, t)
                    nb = int(l1ch[t])
                    et = epool.tile([P, l1max * D_IN], L1D, tag="etL1")
                    nc.sync.dma_start(
                        et[:, 0 : nb * D_IN],
                        x1g[:, int(l1off[t]) * P : (int(l1off[t]) + nb) * P],
                    )
                    return [(et, nb)]

                # prime group 0's edge data + bsel/xo
                calls_cur = {t: issue_edges(t) for t in grs[0]}
                pre_cur = {t: load_pre(lay, t) for t in grs[0]}

                for gi, gts in enumerate(grs):
                    calls_g = calls_cur
                    pre = pre_cur
                    if gi + 1 < NG:
                        calls_cur = {t: issue_edges(t) for t in grs[gi + 1]}
                        pre_cur = {t: load_pre(lay, t) for t in grs[gi + 1]}

                    nodes_c = sum(tile_rows(t) for t in gts)

                    # -- scatter-add matmuls per tile
                    h_tiles = []
                    for t in gts:
                        bs, xo = pre[t]
                        aggps = agg_ps.tile([P, din], F32, tag="agg")
                        if lay == 0:
                            nb = int(l1ch[t])
                            bsr = bs[:, 0 : nb * P].rearrange("p (s j) -> p s j", j=P)
                            et, _ = calls_g[t][0]
                            etr = et[:, 0 : nb * D_IN].rearrange(
                                "p (s e) -> p s e", e=D_IN
                            )
                            if FP8L1:
                                for k in range(nb // 2):
                                    nc.tensor.matmul(
                                        aggps[:],
                                        lhsT=bsr[:, 2 * k : 2 * k + 2, :],
                                        rhs=etr[:, 2 * k : 2 * k + 2, :],
                                        start=(k == 0),
                                        stop=(k == nb // 2 - 1),
                                        perf_mode=DR,
                                    )
                            else:
                                for k in range(nb):
                                    nc.tensor.matmul(
                                        aggps[:],
                                        lhsT=bsr[:, k, :],
                                        rhs=etr[:, k, :],
                                        start=(k == 0),
                                        stop=(k == nb - 1),
                                    )
                        else:
                            ncht_t = int(ncht[t])
                            bsr = bs[:, 0 : ncht_t * P].rearrange(
                                "p (s j) -> p s j", j=P
                            )
                            k = 0
                            for et, npiece in calls_g[t]:
                                etr = et[:, 0 : npiece * H].rearrange(
                                    "p (s e) -> p s e", e=H
                                )
                                for j in range(npiece):
                                    nc.tensor.matmul(
                                        aggps[:],
                                        lhsT=bsr[:, k, :],
                                        rhs=etr[:, j, :],
                                        start=(k == 0),
                                        stop=(k == ncht_t - 1),
                                    )
                                    k += 1
                        # self term fused into the PSUM drain (h/4 domain)
                        h_sb = hpool.tile([P, din], BF16, tag="h")
                        nc.vector.tensor_tensor(
                            out=h_sb[:],
                            in0=aggps[:],
                            in1=xo[:],
                            op=mybir.AluOpType.add,
                        )
                        h_tiles.append(h_sb)

                    # transpose h -> feature-major, then cast to the MLP dtype
                    hfm = fpool.tile([P, fch * 512], BF16, tag="hfm")
                    for ti, t in enumerate(gts):
                        tps = tp_ps.tile([P, fch * P], BF16, tag="tp")
                        for f in range(fch):
                            nc.tensor.transpose(
                                out=tps[:, f * P : (f + 1) * P],
                                in_=h_tiles[ti][:, f * P : (f + 1) * P],
                                identity=id16_sb[:],
                            )
                        nc.vector.tensor_copy(
                            hfm[:, 0 : fch * 512].rearrange("p (f n) -> p f n", n=512)[
                                :, :, ti * P : (ti + 1) * P
                            ],
                            tps[:, 0 : fch * P].rearrange("p (f j) -> p f j", j=P),
                        )
                    if FP8MLP:
                        hfm8 = fpool.tile([P, fch * 512], F8, tag="hfm8")
                        nc.vector.tensor_copy(
                            hfm8[:, 0 : fch * 512].rearrange(
                                "p (f n) -> p f n", n=512
                            )[:, :, 0:nodes_c],
                            hfm[:, 0 : fch * 512].rearrange(
                                "p (f n) -> p f n", n=512
                            )[:, :, 0:nodes_c],
                        )
                        hin = hfm8
                    else:
                        hin = hfm
                    hinr = hin[:, 0 : fch * 512].rearrange("p (f n) -> p f n", n=512)
                    w1r = w1t_sb[:, 0 : fch * H].rearrange("p (f o) -> p f o", o=H)

                    # MLP1: z = relu(h @ W1T + b1), feature-major
                    z8 = zpool.tile([P, 4 * 512], WD, tag="z8")
                    z8r = z8[:, 0 : 4 * 512].rearrange("p (f n) -> p f n", n=512)
                    for fo in range(4):
                        zps = z_ps.tile([P, 512], F32, tag="z")
                        if FP8MLP and fch > 1:
                            for m in range(fch // 2):
                                nc.tensor.matmul(
                                    zps[:, :nodes_c],
                                    lhsT=w1r[:, 2 * m : 2 * m + 2, fo * P : (fo + 1) * P],
                                    rhs=hinr[:, 2 * m : 2 * m + 2, 0:nodes_c],
                                    start=(m == 0),
                                    stop=(m == fch // 2 - 1),
                                    perf_mode=DR,
                                )
                        else:
                            for fi in range(fch):
                                nc.tensor.matmul(
                                    zps[:, :nodes_c],
                                    lhsT=w1r[:, fi, fo * P : (fo + 1) * P],
                                    rhs=hinr[:, fi, 0:nodes_c],
                                    start=(fi == 0),
                                    stop=(fi == fch - 1),
                                )
                        nc.scalar.activation(
                            z8r[:, fo, 0:nodes_c],
                            zps[:, :nodes_c],
                            mybir.ActivationFunctionType.Relu,
                            bias=b1_sb[:, lay * 4 + fo : lay * 4 + fo + 1],
                        )

                    # MLP2: h_next = z @ W2T + b2, node-major
                    w2r = w2t_sb[:, 0 : 4 * H].rearrange("p (f o) -> p f o", o=H)
                    for ti, t in enumerate(gts):
                        rows = tile_rows(t)
                        h2ps = h2_ps.tile([P, H], F32, tag="h2")
                        if FP8MLP:
                            for m in range(2):
                                nc.tensor.matmul(
                                    h2ps[:rows, :],
                                    lhsT=z8r[:, 2 * m : 2 * m + 2, ti * P : ti * P + rows],
                                    rhs=w2r[:, 2 * m : 2 * m + 2, :],
                                    start=(m == 0),
                                    stop=(m == 1),
                                    perf_mode=DR,
                                )
                        else:
                            for zf in range(4):
                                nc.tensor.matmul(
                                    h2ps[:rows, :],
                                    lhsT=z8r[:, zf, ti * P : ti * P + rows],
                                    rhs=w2r[:, zf, :],
                                    start=(zf == 0),
                                    stop=(zf == 3),
                                )
                        if lay < N_LAYERS - 1:
                            agt = agpool.tile([P, H], BF16, tag="ag")
                            nc.vector.tensor_tensor(
                                out=agt[:rows, :],
                                in0=h2ps[:rows, :],
                                in1=b2_sb[:rows, :],
                                op=mybir.AluOpType.add,
                            )
                            bt = next(
                                bi for bi, (s0, e0) in enumerate(BANKS)
                                if s0 <= t * P < e0
                            )
                            o = t * P - BANKS[bt][0]
                            nc.sync.dma_start(
                                agx[lay % 2][bt][o : o + rows, :], agt[:rows, :]
                            )
                        else:
                            hn = agpool.tile([P, H], BF16, tag="hn")
                            nc.vector.tensor_tensor(
                                out=hn[:rows, :],
                                in0=h2ps[:rows, :],
                                in1=b2_sb[:rows, :],
                                op=mybir.AluOpType.add,
                            )
                            nc.tensor.matmul(
                                poolps[:],
                                lhsT=pool_sb[:rows, t * G : (t + 1) * G],
                                rhs=hn[:rows, :],
                                start=(t == 0),
                                stop=(t == TILES - 1),
                            )

                    # split AllGather: each bank fires as soon as its tiles are done
                    if lay < N_LAYERS - 1:
                        for b in range(NBANKS):
                            bank_done = cdiv(BANKS[b][1], P) - 1
                            if bank_done not in gts:
                                continue
                            agt_, xft_ = agx[lay % 2][b], xfx[lay % 2][b]
                            if _no_cc():
                                nc.sync.dma_start(
                                    xft_[0 : agt_.shape[0], :], agt_[:, :]
                                )
                            else:
                                nc.gpsimd.collective_compute(
                                    "AllGather",
                                    mybir.AluOpType.bypass,
                                    replica_groups=rg,
                                    ins=[agt_[:, :]],
                                    outs=[xft_[:, :]],
                                )

            # ---- pooled epilogue (replicated on every core)
            poolsb = cpool.tile([G, H], F32)
            nc.vector.tensor_copy(poolsb[:], poolps[:])
            nc.sync.dma_start(prb[:, :], poolsb[:])
            if _no_cc():
                nc.sync.dma_start(pro[:, :], prb[:, :])
            else:
                nc.gpsimd.collective_compute(
                    "AllReduce",
                    mybir.AluOpType.add,
                    replica_groups=rg,
                    ins=[prb[:, :]],
                    outs=[pro[:, :]],
                )
            pr_sb = cpool.tile([G, H], F32)
            nc.sync.dma_start(pr_sb[:], pro[:, :])
            nc.vector.tensor_scalar_mul(pr_sb[:], pr_sb[:], cinv_sb[:, 0:1])
            tmp = cpool.tile([G, H], F32)
            nc.vector.tensor_tensor(
                out=tmp[:], in0=pr_sb[:], in1=fcw_sb[:], op=mybir.AluOpType.mult
            )
            dot = cpool.tile([G, 1], F32)
            nc.vector.tensor_reduce(
                out=dot[:], in_=tmp[:], axis=mybir.AxisListType.X, op=mybir.AluOpType.add
            )
            osb = cpool.tile([G, 1], F32)
            nc.scalar.activation(
                osb[:],
                dot[:],
                mybir.ActivationFunctionType.Sigmoid,
                bias=fcb_sb[:, 0:1],
            )
            nc.sync.dma_start(out_ext[:, :], osb[:])

    nc.compile()
    return nc


# ---------------- host wrapper ------------------------------------------------
def _prepare_inputs(x, edge_index, batch, w1_0, b1_0, w2_0, b2_0,
                    w1_rest, b1_rest, w2_rest, b2_rest, fc_w, fc_b):
    # activations live in the h/4 domain on device (exact bf16 exponent shift);
    # W1 is scaled by 4, W2 by 1/4 (except the last layer, which feeds the
    # pool at true scale), so all MLP tensors stay inside fp8 e4m3 range.
    S = np.float32(4.0)
    x0q = np.asarray(x, np.float32) / S
    nch, l1ch, idx16, dlocs, x1g = _preprocess_edges(np.asarray(edge_index), x0q)
    pool, cinv = _build_pool_onehot(batch)
    l1max = int(np.asarray(l1ch).max())

    nwd = nf8 if FP8MLP else nbf16
    w1tl = [_spectral_normalize(w1_0).T * S]
    w2tl = [_spectral_normalize(w2_0).T / S]
    b1l = [np.asarray(b1_0, np.float32)]
    b2l = [np.asarray(b2_0, np.float32) / S]
    for i in range(N_LAYERS - 1):
        last = i == N_LAYERS - 2
        w1tl.append(_spectral_normalize(w1_rest[i]).T * S)
        w2tl.append(_spectral_normalize(w2_rest[i]).T * (np.float32(1.0) if last else 1.0 / S))
        b1l.append(np.asarray(b1_rest[i], np.float32))
        b2l.append(np.asarray(b2_rest[i], np.float32) * (np.float32(1.0) if last else 1.0 / S))

    w1t0_np = np.ascontiguousarray(w1tl[0])                      # [128, 512]
    w1tr_np = np.ascontiguousarray(np.concatenate(w1tl[1:], 0))  # [3*512, 512]
    w2t_np = np.ascontiguousarray(np.concatenate(w2tl, 0))       # [4*512, 512]
    b1c_np = np.zeros((P, N_LAYERS * 4), np.float32)
    for l in range(N_LAYERS):
        for f in range(4):
            b1c_np[:, l * 4 + f] = b1l[l][f * P : (f + 1) * P]
    b2bc_np = np.zeros((N_LAYERS * P, H), np.float32)
    for l in range(N_LAYERS):
        b2bc_np[l * P : (l + 1) * P, :] = b2l[l][None, :]

    iota_np = np.tile(np.arange(P, dtype=np.float32), l1max)[None, :].repeat(P, 0)
    shared = {
        "w1t0": w1t0_np.astype(nwd),
        "w1tr": w1tr_np.astype(nwd),
        "w2t": w2t_np.astype(nwd),
        "b1c": b1c_np,
        "b2bc": b2bc_np,
        "iotar": iota_np.astype(nbf16),
        "ident16": np.eye(P, dtype=np.float32).astype(nbf16),
        "cinv": cinv[:, None],
        "fcwb": np.repeat(np.asarray(fc_w, np.float32), G, axis=0),
        "fcb": np.full((G, 1), np.float32(np.asarray(fc_b).reshape(-1)[0]), np.float32),
    }
    x0q16 = x0q.astype(nbf16)
    in_maps = []
    for c in range(CORES):
        m = dict(shared)
        m["xown0"] = np.ascontiguousarray(x0q16[c * NPC : (c + 1) * NPC])
        m["x1g"] = np.ascontiguousarray(x1g[c]) if FP8L1 else np.ascontiguousarray(
            x1g[c]).astype(nbf16)
        m["idx16"] = np.ascontiguousarray(idx16[c])
        m["dlocs"] = np.ascontiguousarray(dlocs[c])
        m["pool1h"] = np.ascontiguousarray(pool[c]).astype(nbf16)
        in_maps.append(m)
    return nch, l1ch, in_maps


_prog_cache = {}
last_results = None


def kernel(x, edge_index, batch, w1_0, b1_0, w2_0, b2_0,
           w1_rest, b1_rest, w2_rest, b2_rest, fc_w, fc_b, **run_kwargs):
    global last_results
    nch, l1ch, in_maps = _prepare_inputs(
        x, edge_index, batch, w1_0, b1_0, w2_0, b2_0,
        w1_rest, b1_rest, w2_rest, b2_rest, fc_w, fc_b,
    )
    key = np.asarray(nch).tobytes() + np.asarray(l1ch).tobytes()
    if key not in _prog_cache:
        _prog_cache[key] = build_program(nch, l1ch)
    nc = _prog_cache[key]
    res = run_bass_kernel_spmd(nc, in_maps, core_ids=list(range(CORES)), **run_kwargs)
    last_results = res
    return np.asarray(res.results[0]["out"], np.float32)


# revision 25
# speedup vs baseline: 1.4195x; 1.0012x over previous
"""GIN discriminator (4-layer GINConv + global mean pool + sigmoid) on 8 trn2 cores.

Sharding: nodes are split contiguously across 8 cores (6250 each). Each layer:
  - activations of all nodes are replicated per-core in DRAM (bf16, scaled h/4),
    via a 2-bank split AllGather (banks fire as their tiles finish)
  - each core gathers edge-source rows (bf16, 1KB elems — bandwidth-bound) for
    edges whose dst it owns (dma_gather, <=6-chunk calls: HW desc-ring cap),
    scatter-adds them per 128-dst tile with one-hot matmuls into PSUM
  - the self term x_i is fused into the PSUM->SBUF drain as a DVE add
  - the MLP runs in fp8 e4m3 with DoubleRow pairing (0.5 cyc/row): activations
    are stored as h/4 everywhere (exact bf16 exponent shift), W1 is scaled by
    4 and W2 by 1/4 on the host, so all fp8 tensors stay well inside e4m3
    range and the arithmetic is exact up to fp8 rounding
  - layer 1 aggregation also runs fp8 DoubleRow from host-pre-gathered x/4
Pooling: per-core partial graph sums via one-hot matmul, AllReduce, then
counts/fc/sigmoid replicated on every core. Spectral norm of the weights and
all edge bucketing run on the host in numpy.
"""

import numpy as np
import ml_dtypes

import concourse.bass as bass
import concourse.bacc as bacc
import concourse.mybir as mybir
import concourse.tile as tile
from concourse.bass_utils import run_bass_kernel_spmd

BF16 = mybir.dt.bfloat16
F32 = mybir.dt.float32
F8 = mybir.dt.float8e4
I16 = mybir.dt.int16
nbf16 = ml_dtypes.bfloat16
nf8 = ml_dtypes.float8_e4m3

# ---------------- problem config (hardcoded for the graded problem) ----------
CORES = 8
N = 50000
E = 800000
G = 64
D_IN = 128
H = 512
N_LAYERS = 4
SN_ITERS = 5

P = 128          # partitions

import os as _os

G4 = int(_os.environ.get("KBASS_G4", "4"))          # tiles per compute group
MAX_GATHER_CHUNKS = int(_os.environ.get("KBASS_MAXCH", "6"))  # HW ring cap ~1008 idx/call
N_SWDGE_QUEUES = int(_os.environ.get("KBASS_NSWQ", "4"))
SWDGE_SCRATCH = int(_os.environ.get("KBASS_SCRATCH", "16384"))
EDGE_BUFS = int(_os.environ.get("KBASS_EBUFS", "9"))
FP8MLP = _os.environ.get("KBASS_FP8MLP", "1") == "1"
FP8L1 = _os.environ.get("KBASS_FP8L1", "1") == "1"


def _bank_geometry(npc, tiles):
    """Tile-aligned bank splits (per-rank row ranges) for the split AllGather."""
    if tiles >= 2:
        tsplits = [(tiles + 1) // 2, tiles]
    else:
        tsplits = [tiles]
    starts = [0] + [min(t * P, npc) for t in tsplits]
    return [(starts[i], starts[i + 1]) for i in range(len(tsplits))]


NPC = N // CORES                      # nodes per core
TILES = -(-NPC // P)                  # dst tiles per core
LAST_ROWS = NPC - (TILES - 1) * P     # rows in the last tile
BANKS = _bank_geometry(NPC, TILES)    # [(row_start, row_end) per rank]
NBANKS = len(BANKS)


def cdiv(a, b):
    return -(-a // b)


def groups_list():
    return [list(range(g, min(g + G4, TILES))) for g in range(0, TILES, G4)]


def _no_cc():
    return _os.environ.get("KBASS_NO_CC", "0") == "1"


def _patch_tile_swdge_lanes():
    """Partition Tile's 8 DMASW completion-sem lanes by SWDGE queue instead of
    global round-robin (the default can put DMAs from different queues on one
    lane, breaking the per-lane FIFO-completion invariant Tile assumes)."""
    import concourse.tile_sem_assignment as tsa
    from concourse.tile_scheduler import DMAInst

    if getattr(tsa.TileClockTick, "_kbass_qaware", False):
        return
    orig = tsa.TileClockTick._assign_tick

    def _assign_tick(self, inst):
        if (
            isinstance(inst, DMAInst)
            and inst.engine == mybir.EngineType.Pool
            and not isinstance(inst, bass_isa.UserSyncedRemoteDMADescs)
        ):
            q = getattr(inst, "queue_num", 0) or 0
            lanes_per_q = max(1, self.swdge_sem_count // N_SWDGE_QUEUES)
            if not hasattr(self, "_kbass_qtog"):
                self._kbass_qtog = {}
            tog = self._kbass_qtog.get(q, 0)
            self._kbass_qtog[q] = (tog + 1) % lanes_per_q
            self.next_sw_dma_idx = (q * lanes_per_q + tog) % self.swdge_sem_count
        return orig(self, inst)

    tsa.TileClockTick._assign_tick = _assign_tick
    tsa.TileClockTick._kbass_qaware = True


def configure(n=50000, e=800000, g=64, d_in=128, h=512, n_layers=4):
    """Reconfigure module geometry (used by test harnesses for small smoke runs)."""
    global N, E, G, D_IN, H, N_LAYERS, NPC, TILES, LAST_ROWS, BANKS, NBANKS
    N, E, G, D_IN, H, N_LAYERS = n, e, g, d_in, h, n_layers
    NPC = N // CORES
    TILES = -(-NPC // P)
    LAST_ROWS = NPC - (TILES - 1) * P
    BANKS = _bank_geometry(NPC, TILES)
    NBANKS = len(BANKS)
    _prog_cache.clear()


def tile_rows(t):
    return LAST_ROWS if t == TILES - 1 else P


# ---------------- host-side math ---------------------------------------------
def _spectral_normalize(W):
    W = np.asarray(W, np.float32)
    u = np.ones((W.shape[0],), np.float32) / np.float32(np.sqrt(np.float32(W.shape[0])))
    for _ in range(SN_ITERS):
        v = W.T @ u
        v = v / (np.linalg.norm(v) + np.float32(1e-12))
        u = W @ v
        u = u / (np.linalg.norm(u) + np.float32(1e-12))
    sigma = u @ (W @ v)
    return (W / sigma).astype(np.float32)


def _pack_call(idx, n_chunks):
    """int16 idxs for one dma_gather call: index i lives at [i%16, i//16],
    replicated across the eight 16-partition groups (one per Q7 core)."""
    L = np.zeros((n_chunks * P,), np.int16)
    L[: len(idx)] = idx.astype(np.int16)
    return np.tile(L.reshape(-1, 16).T, (8, 1))  # [128, n_chunks*8]


def _preprocess_edges(edge_index, x0q):
    """Bucket edges by (dst core, dst tile, src bank); uniform chunk counts.

    Returns:
      nch    [TILES, NBANKS] per-(tile,bank) 128-edge chunk counts (max/cores)
      l1ch   [TILES] per-tile chunk count rounded up to even (layer-1 DR pairs;
             the pad chunk has an all-zero one-hot)
      idx16  [CORES, 128, tot_ch*8] gather idx packed per (t, b, <=6ch piece)
      dlocs  [CORES, 128, l1tot] bf16 dst slots (-1 pads) in tile-major order
      x1g    [CORES, 128, l1tot*128] fp8 layer-1 pre-gathered x/4 edge feats
    """
    src = np.asarray(edge_index[0], np.int64)
    dst = np.asarray(edge_index[1], np.int64)
    core = dst // NPC
    tloc = (dst % NPC) // P
    dloc = (dst % NPC) % P
    r = src // NPC
    i = src % NPC
    bstarts = np.array([b[0] for b in BANKS] + [NPC], np.int64)
    bank = np.searchsorted(bstarts, i, side="right") - 1
    brows = bstarts[1:] - bstarts[:-1]
    srcloc = r * brows[bank] + (i - bstarts[bank])

    key = (core * TILES + tloc) * NBANKS + bank
    # secondary sort by srcloc: ascending gather addresses within each bucket
    # (better HBM locality for the 1KB random reads)
    order = np.lexsort((srcloc, key))
    key_s, srcloc_s, dloc_s, src_s = key[order], srcloc[order], dloc[order], src[order]
    counts = np.bincount(key_s, minlength=CORES * TILES * NBANKS).reshape(
        CORES, TILES, NBANKS
    )
    starts = np.zeros(CORES * TILES * NBANKS + 1, np.int64)
    np.cumsum(counts.reshape(-1), out=starts[1:])

    nch = np.maximum(cdiv(counts.max(axis=0), P), 1)  # [TILES, NBANKS]
    ncht = nch.sum(axis=1)                            # [TILES]
    tot_ch = int(ncht.sum())
    l1ch = ncht + (ncht % 2)                          # even for L1 DR pairs
    l1off = np.zeros(TILES + 1, np.int64)
    np.cumsum(l1ch, out=l1off[1:])
    l1tot = int(l1off[-1])

    idx16 = np.zeros((CORES, P, tot_ch * 8), np.int16)
    dlocs = np.full((CORES, P, l1tot), -1.0, nbf16)
    x1g = np.zeros((CORES, P, l1tot * P), nf8)
    for c in range(CORES):
        icol = 0
        for t in range(TILES):
            for b in range(NBANKS):
                k = (c * TILES + t) * NBANKS + b
                s, e = starts[k], starts[k + 1]
                nchb = int(nch[t, b])
                bidx = np.zeros((nchb * P,), np.int64)
                bidx[: e - s] = srcloc_s[s:e]
                done = 0
                while done < nchb:
                    npiece = min(MAX_GATHER_CHUNKS, nchb - done)
                    idx16[c, :, icol : icol + npiece * 8] = _pack_call(
                        bidx[done * P : (done + npiece) * P], npiece
                    )
                    icol += npiece * 8
                    done += npiece
        for t in range(TILES):
            dcol = int(l1off[t])
            for b in range(NBANKS):
                k = (c * TILES + t) * NBANKS + b
                s, e = starts[k], starts[k + 1]
                nchb = int(nch[t, b])
                dl = np.full((nchb * P,), -1.0, np.float32)
                dl[: e - s] = dloc_s[s:e]
                dlocs[c, :, dcol : dcol + nchb] = dl.reshape(nchb, P).T.astype(nbf16)
                gsrc = np.zeros((nchb * P,), np.int64)
                gsrc[: e - s] = src_s[s:e]
                x1g[c, :, dcol * P : (dcol + nchb) * P] = (
                    x0q[gsrc]
                    .reshape(nchb, P, D_IN)
                    .transpose(1, 0, 2)
                    .reshape(P, nchb * D_IN)
                )
                dcol += nchb
    return nch, l1ch, idx16, dlocs, x1g


def _build_pool_onehot(batch):
    batch = np.asarray(batch, np.int64)
    pool = np.zeros((CORES, P, TILES * G), np.float32)
    for c in range(CORES):
        b = batch[c * NPC : (c + 1) * NPC]
        for i in range(NPC):
            t, p = i // P, i % P
            pool[c, p, t * G + int(b[i])] = 1.0
    counts = np.bincount(batch, minlength=G).astype(np.float32)
    cinv = (1.0 / np.maximum(counts, 1.0)).astype(np.float32)
    return pool, cinv


# ---------------- device program ---------------------------------------------
from concourse import bass_isa


def build_program(nch, l1ch):
    _patch_tile_swdge_lanes()
    nch = np.asarray(nch)
    l1ch = np.asarray(l1ch)
    ncht = nch.sum(axis=1)
    ncht_max = int(ncht.max())
    l1max = int(l1ch.max())
    l1off = np.zeros(TILES + 1, np.int64)
    np.cumsum(l1ch, out=l1off[1:])
    l1tot = int(l1off[-1])
    grs = groups_list()
    NG = len(grs)
    icol_off = np.zeros((TILES, NBANKS), np.int64)
    acc = 0
    for t in range(TILES):
        for b in range(NBANKS):
            icol_off[t, b] = acc
            acc += int(nch[t, b]) * 8
    idx_cols = acc
    WD = F8 if FP8MLP else BF16  # MLP weight/act dtype
    L1D = F8 if FP8L1 else BF16  # layer-1 edge dtype
    DR = mybir.MatmulPerfMode.DoubleRow

    nc = bacc.Bacc(
        num_devices=CORES,
        target_bir_lowering=False,
        debug=False,
        num_swdge_queues=N_SWDGE_QUEUES,
        dynamic_dma_scratch_size=SWDGE_SCRATCH,
    )

    # ---- external inputs
    x1g = nc.declare_dram_parameter("x1g", [P, l1tot * P], L1D, isOutput=False)
    xown0 = nc.declare_dram_parameter("xown0", [NPC, D_IN], BF16, isOutput=False)
    idx16 = nc.declare_dram_parameter("idx16", [P, idx_cols], I16, isOutput=False)
    dlocs = nc.declare_dram_parameter("dlocs", [P, l1tot], BF16, isOutput=False)
    iotar = nc.declare_dram_parameter("iotar", [P, l1max * P], BF16, isOutput=False)
    pool1h = nc.declare_dram_parameter("pool1h", [P, TILES * G], BF16, isOutput=False)
    w1t0 = nc.declare_dram_parameter("w1t0", [D_IN, H], WD, isOutput=False)
    w1tr = nc.declare_dram_parameter("w1tr", [(N_LAYERS - 1) * H, H], WD, isOutput=False)
    w2t = nc.declare_dram_parameter("w2t", [N_LAYERS * H, H], WD, isOutput=False)
    b1c = nc.declare_dram_parameter("b1c", [P, N_LAYERS * 4], F32, isOutput=False)
    b2bc = nc.declare_dram_parameter("b2bc", [N_LAYERS * P, H], F32, isOutput=False)
    ident16 = nc.declare_dram_parameter("ident16", [P, P], BF16, isOutput=False)
    cinv = nc.declare_dram_parameter("cinv", [G, 1], F32, isOutput=False)
    fcwb = nc.declare_dram_parameter("fcwb", [G, H], F32, isOutput=False)
    fcb = nc.declare_dram_parameter("fcb", [G, 1], F32, isOutput=False)
    out_ext = nc.declare_dram_parameter("out", [G, 1], F32, isOutput=True)

    # ---- internal DRAM (double-buffered per layer parity)
    agx = [
        [
            nc.dram_tensor(f"ag{b}_{i}", [BANKS[b][1] - BANKS[b][0], H], BF16)
            for b in range(NBANKS)
        ]
        for i in range(2)
    ]
    xfx = [
        [
            nc.dram_tensor(
                f"xf{b}_{i}",
                [CORES * (BANKS[b][1] - BANKS[b][0]), H],
                BF16,
                addr_space="Shared",
            )
            for b in range(NBANKS)
        ]
        for i in range(2)
    ]
    prb = nc.dram_tensor("prb", [G, H], F32)
    pro = nc.dram_tensor("pro", [G, H], F32, addr_space="Shared")

    rg = [list(range(CORES))]

    with tile.TileContext(nc) as tc:
        with (
            tc.tile_pool(name="consts", bufs=1) as cpool,
            tc.tile_pool(name="wts", bufs=2) as wpool,
            tc.tile_pool(name="edge", bufs=EDGE_BUFS) as epool,
            tc.tile_pool(name="bsel", bufs=6) as bpool,
            tc.tile_pool(name="xo", bufs=2 * G4) as xopool,
            tc.tile_pool(name="hsb", bufs=5) as hpool,
            tc.tile_pool(name="hfm", bufs=2) as fpool,
            tc.tile_pool(name="zt", bufs=3) as zpool,
            tc.tile_pool(name="agt", bufs=4) as agpool,
            tc.tile_pool(name="ps_agg", bufs=2, space="PSUM") as agg_ps,
            tc.tile_pool(name="ps_tp", bufs=1, space="PSUM") as tp_ps,
            tc.tile_pool(name="ps_z", bufs=2, space="PSUM") as z_ps,
            tc.tile_pool(name="ps_h2", bufs=2, space="PSUM") as h2_ps,
            tc.tile_pool(name="ps_pool", bufs=1, space="PSUM") as pool_ps,
        ):
            # ---- load constants
            idx_sb = cpool.tile([P, idx_cols], I16)
            nc.sync.dma_start(idx_sb[:], idx16[:, :])
            dloc_sb = cpool.tile([P, l1tot], BF16)
            nc.sync.dma_start(dloc_sb[:], dlocs[:, :])
            iota_sb = cpool.tile([P, l1max * P], BF16)
            nc.sync.dma_start(iota_sb[:], iotar[:, :])
            id16_sb = cpool.tile([P, P], BF16)
            nc.sync.dma_start(id16_sb[:], ident16[:, :])
            b1_sb = cpool.tile([P, N_LAYERS * 4], F32)
            nc.sync.dma_start(b1_sb[:], b1c[:, :])
            cinv_sb = cpool.tile([G, 1], F32)
            nc.sync.dma_start(cinv_sb[:], cinv[:, :])
            fcw_sb = cpool.tile([G, H], F32)
            nc.sync.dma_start(fcw_sb[:], fcwb[:, :])
            fcb_sb = cpool.tile([G, 1], F32)
            nc.sync.dma_start(fcb_sb[:], fcb[:, :])
            pool_sb = cpool.tile([P, TILES * G], BF16)
            nc.sync.dma_start(pool_sb[:], pool1h[:, :])

            self_qn = [0]  # rotating SWDGE queue assignment for gathers

            def issue_gathers(lay, t):
                """Per-(tile, bank) dma_gather pieces (<= MAX_GATHER_CHUNKS).

                Returns [(et_tile, n_chunks)] in tile-major chunk order."""
                banks_src = [t_[:, :] for t_ in xfx[(lay - 1) % 2]]
                calls = []
                for b in range(NBANKS):
                    nchb = int(nch[t, b])
                    icol = int(icol_off[t, b])
                    done = 0
                    while done < nchb:
                        npiece = min(MAX_GATHER_CHUNKS, nchb - done)
                        et = epool.tile(
                            [P, MAX_GATHER_CHUNKS * H], BF16, tag="etile"
                        )
                        nc.gpsimd.dma_gather(
                            out_ap=et[:, 0 : npiece * H].rearrange(
                                "p (s e) -> p s e", e=H
                            ),
                            in_ap=banks_src[b],
                            idxs_ap=idx_sb[:, icol : icol + npiece * 8],
                            num_idxs=npiece * P,
                            num_idxs_reg=npiece * P,
                            elem_size=H,
                            queue_num=self_qn[0] % N_SWDGE_QUEUES,
                        )
                        self_qn[0] += 1
                        calls.append((et, npiece))
                        icol += npiece * 8
                        done += npiece
                return calls

            def load_pre(lay, t):
                """bsel one-hot (DVE is_equal) + self-term xo for tile t."""
                rows = tile_rows(t)
                nb = int(l1ch[t]) if lay == 0 else int(ncht[t])
                dcol = int(l1off[t])
                if lay == 0:
                    bs = bpool.tile([P, l1max * P], L1D, tag="bs0")
                else:
                    bs = bpool.tile([P, l1max * P], BF16, tag="bs")
                nc.vector.tensor_tensor(
                    out=bs[:, 0 : nb * P].rearrange("p (s j) -> p s j", j=P),
                    in0=iota_sb[:, 0 : nb * P].rearrange("p (s j) -> p s j", j=P),
                    in1=dloc_sb[:, dcol : dcol + nb, None].broadcast_to([P, nb, P]),
                    op=mybir.AluOpType.is_equal,
                )
                if lay == 0:
                    xo = xopool.tile([P, D_IN], BF16, tag="xo0")
                    if rows < P:
                        nc.vector.memset(xo[:], 0.0)
                    nc.sync.dma_start(xo[:rows, :], xown0[t * P : t * P + rows, :])
                else:
                    xo = xopool.tile([P, H], BF16, tag="xo")
                    if rows < P:
                        nc.vector.memset(xo[:], 0.0)
                    bt = next(
                        bi for bi, (s0, e0) in enumerate(BANKS) if s0 <= t * P < e0
                    )
                    o = t * P - BANKS[bt][0]
                    nc.sync.dma_start(
                        xo[:rows, :], agx[(lay - 1) % 2][bt][o : o + rows, :]
                    )
                return bs, xo

            for lay in range(N_LAYERS):
                din = D_IN if lay == 0 else H
                fch = din // P  # feature chunks of the layer input

                # per-layer weights (bufs=2 -> prefetch overlaps prev layer)
                w1t_sb = wpool.tile([P, 4 * H], WD, tag="w1t")
                if lay == 0:
                    nc.sync.dma_start(w1t_sb[:, 0:H], w1t0[:, :])
                else:
                    for fi in range(fch):
                        nc.sync.dma_start(
                            w1t_sb[:, fi * H : (fi + 1) * H],
                            w1tr[(lay - 1) * H + fi * P : (lay - 1) * H + (fi + 1) * P, :],
                        )
                w2t_sb = wpool.tile([P, 4 * H], WD, tag="w2t")
                for zf in range(4):
                    nc.sync.dma_start(
                        w2t_sb[:, zf * H : (zf + 1) * H],
                        w2t[lay * H + zf * P : lay * H + (zf + 1) * P, :],
                    )
                b2_sb = wpool.tile([P, H], F32, tag="b2")
                nc.sync.dma_start(b2_sb[:], b2bc[lay * P : (lay + 1) * P, :])

                if lay == N_LAYERS - 1:
                    poolps = pool_ps.tile([G, H], F32)

                def issue_edges(t):
                    if lay > 0:
                        return issue_gathers(lay, t)
                    nb = int(l1ch[t])
                    et = epool.tile([P, l1max * D_IN], L1D, tag="etL1")
                    nc.sync.dma_start(
                        et[:, 0 : nb * D_IN],
                        x1g[:, int(l1off[t]) * P : (int(l1off[t]) + nb) * P],
                    )
                    return [(et, nb)]

                # prime group 0's edge data + bsel/xo
                calls_cur = {t: issue_edges(t) for t in grs[0]}
                pre_cur = {t: load_pre(lay, t) for t in grs[0]}

                for gi, gts in enumerate(grs):
                    calls_g = calls_cur
                    pre = pre_cur
                    if gi + 1 < NG:
                        calls_cur = {t: issue_edges(t) for t in grs[gi + 1]}
                        pre_cur = {t: load_pre(lay, t) for t in grs[gi + 1]}

                    nodes_c = sum(tile_rows(t) for t in gts)

                    # -- scatter-add matmuls per tile
                    h_tiles = []
                    for t in gts:
                        bs, xo = pre[t]
                        aggps = agg_ps.tile([P, din], F32, tag="agg")
                        if lay == 0:
                            nb = int(l1ch[t])
                            bsr = bs[:, 0 : nb * P].rearrange("p (s j) -> p s j", j=P)
                            et, _ = calls_g[t][0]
                            etr = et[:, 0 : nb * D_IN].rearrange(
                                "p (s e) -> p s e", e=D_IN
                            )
                            if FP8L1:
                                for k in range(nb // 2):
                                    nc.tensor.matmul(
                                        aggps[:],
                                        lhsT=bsr[:, 2 * k : 2 * k + 2, :],
                                        rhs=etr[:, 2 * k : 2 * k + 2, :],
                                        start=(k == 0),
                                        stop=(k == nb // 2 - 1),
                                        perf_mode=DR,
                                    )
                            else:
                                for k in range(nb):
                                    nc.tensor.matmul(
                                        aggps[:],
                                        lhsT=bsr[:, k, :],
                                        rhs=etr[:, k, :],
                                        start=(k == 0),
                                        stop=(k == nb - 1),
                                    )
                        else:
                            ncht_t = int(ncht[t])
                            bsr = bs[:, 0 : ncht_t * P].rearrange(
                                "p (s j) -> p s j", j=P
                            )
                            k = 0
                            for et, npiece in calls_g[t]:
                                etr = et[:, 0 : npiece * H].rearrange(
                                    "p (s e) -> p s e", e=H
                                )
                                for j in range(npiece):
                                    nc.tensor.matmul(
                                        aggps[:],
                                        lhsT=bsr[:, k, :],
                                        rhs=etr[:, j, :],
                                        start=(k == 0),
                                        stop=(k == ncht_t - 1),
                                    )
                                    k += 1
                        # self term fused into the PSUM drain (h/4 domain)
                        h_sb = hpool.tile([P, din], BF16, tag="h")
                        nc.vector.tensor_tensor(
                            out=h_sb[:],
                            in0=aggps[:],
                            in1=xo[:],
                            op=mybir.AluOpType.add,
                        )
                        h_tiles.append(h_sb)

                    # transpose h -> feature-major, then cast to the MLP dtype
                    hfm = fpool.tile([P, fch * 512], BF16, tag="hfm")
                    for ti, t in enumerate(gts):
                        tps = tp_ps.tile([P, fch * P], BF16, tag="tp")
                        for f in range(fch):
                            nc.tensor.transpose(
                                out=tps[:, f * P : (f + 1) * P],
                                in_=h_tiles[ti][:, f * P : (f + 1) * P],
                                identity=id16_sb[:],
                            )
                        nc.vector.tensor_copy(
                            hfm[:, 0 : fch * 512].rearrange("p (f n) -> p f n", n=512)[
                                :, :, ti * P : (ti + 1) * P
                            ],
                            tps[:, 0 : fch * P].rearrange("p (f j) -> p f j", j=P),
                        )
                    if FP8MLP:
                        hfm8 = fpool.tile([P, fch * 512], F8, tag="hfm8")
                        nc.vector.tensor_copy(
                            hfm8[:, 0 : fch * 512].rearrange(
                                "p (f n) -> p f n", n=512
                            )[:, :, 0:nodes_c],
                            hfm[:, 0 : fch * 512].rearrange(
                                "p (f n) -> p f n", n=512
                            )[:, :, 0:nodes_c],
                        )
                        hin = hfm8
                    else:
                        hin = hfm
                    hinr = hin[:, 0 : fch * 512].rearrange("p (f n) -> p f n", n=512)
                    w1r = w1t_sb[:, 0 : fch * H].rearrange("p (f o) -> p f o", o=H)

                    # MLP1: z = relu(h @ W1T + b1), feature-major
                    z8 = zpool.tile([P, 4 * 512], WD, tag="z8")
                    z8r = z8[:, 0 : 4 * 512].rearrange("p (f n) -> p f n", n=512)
                    for fo in range(4):
                        zps = z_ps.tile([P, 512], F32, tag="z")
                        if FP8MLP and fch > 1:
                            for m in range(fch // 2):
                                nc.tensor.matmul(
                                    zps[:, :nodes_c],
                                    lhsT=w1r[:, 2 * m : 2 * m + 2, fo * P : (fo + 1) * P],
                                    rhs=hinr[:, 2 * m : 2 * m + 2, 0:nodes_c],
                                    start=(m == 0),
                                    stop=(m == fch // 2 - 1),
                                    perf_mode=DR,
                                )
                        else:
                            for fi in range(fch):
                                nc.tensor.matmul(
                                    zps[:, :nodes_c],
                                    lhsT=w1r[:, fi, fo * P : (fo + 1) * P],
                                    rhs=hinr[:, fi, 0:nodes_c],
                                    start=(fi == 0),
                                    stop=(fi == fch - 1),
                                )
                        nc.scalar.activation(
                            z8r[:, fo, 0:nodes_c],
                            zps[:, :nodes_c],
                            mybir.ActivationFunctionType.Relu,
                            bias=b1_sb[:, lay * 4 + fo : lay * 4 + fo + 1],
                        )

                    # MLP2: h_next = z @ W2T + b2, node-major
                    w2r = w2t_sb[:, 0 : 4 * H].rearrange("p (f o) -> p f o", o=H)
                    for ti, t in enumerate(gts):
                        rows = tile_rows(t)
                        h2ps = h2_ps.tile([P, H], F32, tag="h2")
                        if FP8MLP:
                            for m in range(2):
                                nc.tensor.matmul(
                                    h2ps[:rows, :],
                                    lhsT=z8r[:, 2 * m : 2 * m + 2, ti * P : ti * P + rows],
                                    rhs=w2r[:, 2 * m : 2 * m + 2, :],
                                    start=(m == 0),
                                    stop=(m == 1),
                                    perf_mode=DR,
                                )
                        else:
                            for zf in range(4):
                                nc.tensor.matmul(
                                    h2ps[:rows, :],
                                    lhsT=z8r[:, zf, ti * P : ti * P + rows],
                                    rhs=w2r[:, zf, :],
                                    start=(zf == 0),
                                    stop=(zf == 3),
                                )
                        if lay < N_LAYERS - 1:
                            agt = agpool.tile([P, H], BF16, tag="ag")
                            nc.vector.tensor_tensor(
                                out=agt[:rows, :],
                                in0=h2ps[:rows, :],
                                in1=b2_sb[:rows, :],
                                op=mybir.AluOpType.add,
                            )
                            bt = next(
                                bi for bi, (s0, e0) in enumerate(BANKS)
                                if s0 <= t * P < e0
                            )
                            o = t * P - BANKS[bt][0]
                            nc.sync.dma_start(
                                agx[lay % 2][bt][o : o + rows, :], agt[:rows, :]
                            )
                        else:
                            hn = agpool.tile([P, H], BF16, tag="hn")
                            nc.vector.tensor_tensor(
                                out=hn[:rows, :],
                                in0=h2ps[:rows, :],
                                in1=b2_sb[:rows, :],
                                op=mybir.AluOpType.add,
                            )
                            nc.tensor.matmul(
                                poolps[:],
                                lhsT=pool_sb[:rows, t * G : (t + 1) * G],
                                rhs=hn[:rows, :],
                                start=(t == 0),
                                stop=(t == TILES - 1),
                            )

                    # split AllGather: each bank fires as soon as its tiles are done
                    if lay < N_LAYERS - 1:
                        for b in range(NBANKS):
                            bank_done = cdiv(BANKS[b][1], P) - 1
                            if bank_done not in gts:
                                continue
                            agt_, xft_ = agx[lay % 2][b], xfx[lay % 2][b]
                            if _no_cc():
                                nc.sync.dma_start(
                                    xft_[0 : agt_.shape[0], :], agt_[:, :]
                                )
                            else:
                                nc.gpsimd.collective_compute(
                                    "AllGather",
                                    mybir.AluOpType.bypass,
                                    replica_groups=rg,
                                    ins=[agt_[:, :]],
                                    outs=[xft_[:, :]],
                                )

            # ---- pooled epilogue (replicated on every core)
            poolsb = cpool.tile([G, H], F32)
            nc.vector.tensor_copy(poolsb[:], poolps[:])
            nc.sync.dma_start(prb[:, :], poolsb[:])
            if _no_cc():
                nc.sync.dma_start(pro[:, :], prb[:, :])
            else:
                nc.gpsimd.collective_compute(
                    "AllReduce",
                    mybir.AluOpType.add,
                    replica_groups=rg,
                    ins=[prb[:, :]],
                    outs=[pro[:, :]],
                )
            pr_sb = cpool.tile([G, H], F32)
            nc.sync.dma_start(pr_sb[:], pro[:, :])
            nc.vector.tensor_scalar_mul(pr_sb[:], pr_sb[:], cinv_sb[:, 0:1])
            tmp = cpool.tile([G, H], F32)
            nc.vector.tensor_tensor(
                out=tmp[:], in0=pr_sb[:], in1=fcw_sb[:], op=mybir.AluOpType.mult
            )
            dot = cpool.tile([G, 1], F32)
            nc.vector.tensor_reduce(
                out=dot[:], in_=tmp[:], axis=mybir.AxisListType.X, op=mybir.AluOpType.add
            )
            osb = cpool.tile([G, 1], F32)
            nc.scalar.activation(
                osb[:],
                dot[:],
                mybir.ActivationFunctionType.Sigmoid,
                bias=fcb_sb[:, 0:1],
            )
            nc.sync.dma_start(out_ext[:, :], osb[:])

    nc.compile()
    return nc


# ---------------- host wrapper ------------------------------------------------
def _prepare_inputs(x, edge_index, batch, w1_0, b1_0, w2_0, b2_0,
                    w1_rest, b1_rest, w2_rest, b2_rest, fc_w, fc_b):
    # activations live in the h/4 domain on device (exact bf16 exponent shift);
    # W1 is scaled by 4, W2 by 1/4 (except the last layer, which feeds the
    # pool at true scale), so all MLP tensors stay inside fp8 e4m3 range.
    S = np.float32(4.0)
    x0q = np.asarray(x, np.float32) / S
    nch, l1ch, idx16, dlocs, x1g = _preprocess_edges(np.asarray(edge_index), x0q)
    pool, cinv = _build_pool_onehot(batch)
    l1max = int(np.asarray(l1ch).max())

    nwd = nf8 if FP8MLP else nbf16
    w1tl = [_spectral_normalize(w1_0).T * S]
    w2tl = [_spectral_normalize(w2_0).T / S]
    b1l = [np.asarray(b1_0, np.float32)]
    b2l = [np.asarray(b2_0, np.float32) / S]
    for i in range(N_LAYERS - 1):
        last = i == N_LAYERS - 2
        w1tl.append(_spectral_normalize(w1_rest[i]).T * S)
        w2tl.append(_spectral_normalize(w2_rest[i]).T * (np.float32(1.0) if last else 1.0 / S))
        b1l.append(np.asarray(b1_rest[i], np.float32))
        b2l.append(np.asarray(b2_rest[i], np.float32) * (np.float32(1.0) if last else 1.0 / S))

    w1t0_np = np.ascontiguousarray(w1tl[0])                      # [128, 512]
    w1tr_np = np.ascontiguousarray(np.concatenate(w1tl[1:], 0))  # [3*512, 512]
    w2t_np = np.ascontiguousarray(np.concatenate(w2tl, 0))       # [4*512, 512]
    b1c_np = np.zeros((P, N_LAYERS * 4), np.float32)
    for l in range(N_LAYERS):
        for f in range(4):
            b1c_np[:, l * 4 + f] = b1l[l][f * P : (f + 1) * P]
    b2bc_np = np.zeros((N_LAYERS * P, H), np.float32)
    for l in range(N_LAYERS):
        b2bc_np[l * P : (l + 1) * P, :] = b2l[l][None, :]

    iota_np = np.tile(np.arange(P, dtype=np.float32), l1max)[None, :].repeat(P, 0)
    shared = {
        "w1t0": w1t0_np.astype(nwd),
        "w1tr": w1tr_np.astype(nwd),
        "w2t": w2t_np.astype(nwd),
        "b1c": b1c_np,
        "b2bc": b2bc_np,
        "iotar": iota_np.astype(nbf16),
        "ident16": np.eye(P, dtype=np.float32).astype(nbf16),
        "cinv": cinv[:, None],
        "fcwb": np.repeat(np.asarray(fc_w, np.float32), G, axis=0),
        "fcb": np.full((G, 1), np.float32(np.asarray(fc_b).reshape(-1)[0]), np.float32),
    }
    x0q16 = x0q.astype(nbf16)
    in_maps = []
    for c in range(CORES):
        m = dict(shared)
        m["xown0"] = np.ascontiguousarray(x0q16[c * NPC : (c + 1) * NPC])
        m["x1g"] = np.ascontiguousarray(x1g[c]) if FP8L1 else np.ascontiguousarray(
            x1g[c]).astype(nbf16)
        m["idx16"] = np.ascontiguousarray(idx16[c])
        m["dlocs"] = np.ascontiguousarray(dlocs[c])
        m["pool1h"] = np.ascontiguousarray(pool[c]).astype(nbf16)
        in_maps.append(m)
    return nch, l1ch, in_maps


_prog_cache = {}
last_results = None


def kernel(x, edge_index, batch, w1_0, b1_0, w2_0, b2_0,
           w1_rest, b1_rest, w2_rest, b2_rest, fc_w, fc_b, **run_kwargs):
    global last_results
    nch, l1ch, in_maps = _prepare_inputs(
        x, edge_index, batch, w1_0, b1_0, w2_0, b2_0,
        w1_rest, b1_rest, w2_rest, b2_rest, fc_w, fc_b,
    )
    key = np.asarray(nch).tobytes() + np.asarray(l1ch).tobytes()
    if key not in _prog_cache:
        _prog_cache[key] = build_program(nch, l1ch)
    nc = _prog_cache[key]
    res = run_bass_kernel_spmd(nc, in_maps, core_ids=list(range(CORES)), **run_kwargs)
    last_results = res
    return np.asarray(res.results[0]["out"], np.float32)
